# revision 29
# baseline (speedup 1.0000x reference)
"""Trainium2 Bass kernel for the 3-layer GAT (nn_GAT_56341380989571).

Strategy (8 NeuronCores, SPMD):
  - Nodes padded to 100352, sharded contiguously: core k owns 12544 nodes
    (98 blocks of 128). Edges partitioned by receiver; per core, edges are
    bucketed per (128-node block, sender-quarter) with VARIABLE capacity
    (ceil(max-over-cores count / 128) tiles of 128 edge slots), so
    int16-indexed dma_gather calls (one per 7-block group x quarter,
    spread over 4 SWDGE queues) fetch per-edge rows from the quarter's
    shared table tensor.  Variable capacity cuts gather descriptors (the
    SWDGE queue-throughput bottleneck) ~25% vs fixed 3-tile buckets.
  - The per-layer node table [hp | al_s | al_d] (bf16, 512B rows) is
    exchanged via 4 CHUNKED AllGathers per layer (one per sender-quarter,
    (25,25,24,24) blocks each, into its own Shared tensor), fused into the
    edge stage of the previous layer: as soon as a 7-block group's residual
    update lands, the next layer's table rows for those blocks are
    computed, staged and stored; once all blocks of a quarter are stored
    (after groups 3/7/10/13) that quarter is AllGathered while the rest of
    the edge stage continues.
  - hp/al_s/al_d are computed as ONE bf16 matmul pair per block with
    error-compensated split weights (W ~ hi + lo in bf16, accumulated in
    the same PSUM group) to avoid the systematic bf16 weight-rounding
    bias; h itself is stored bf16 (random rounding pools away).
  - Attention: w = exp(lrelu(al_s[s]+al_d[r])) with softmax max-subtraction
    dropped and normalization folded to node level.
  - Segment sums by receiver via indicator matmuls; indicator tiles
    IND[e,p] = (r_rel[e]==p) and transposes IND2 are host-precomputed
    (layer-invariant) and streamed per step as ONE combined DMA on the
    scalar queue.  al_d is expanded edge-wise as IND2 @ al_d_block.
  - Residual add folded into the PE (identity-matmul accumulation onto the
    skip matmul); bias + leaky-relu fused into the single scalar-engine
    PSUM eviction (AF.Prelu + bias), writing h directly in bf16.
  - Graph mean-pool via a mask matmul accumulated during layer 3, then an
    AllReduce of [100,128] partial sums and a redundant tiny MLP.
"""
import os

import numpy as np

import concourse.bacc as bacc
import concourse.mybir as mybir
import concourse.tile as tile
from concourse.bass_utils import run_bass_kernel_spmd
from concourse.library_config import mlp as _mlp_lib

F32 = mybir.dt.float32
BF16 = mybir.dt.bfloat16
I16 = mybir.dt.int16
AF = mybir.ActivationFunctionType
OP = mybir.AluOpType

# problem constants (hardcoded per spec)
N, E, G = 100000, 800000, 100
F_IN, DIM, H, L = 64, 128, 8, 3
HD = DIM // H
SLOPE = 0.2
NCORES = 8
BLK = 128
BPC = 98                 # blocks per core
NPC = BPC * BLK          # 12544 nodes per core
NPAD = NCORES * NPC      # 100352
ELEM = 256               # bf16 elems per table row (512B)
GPB = 7                  # blocks per gather group
NGRP = BPC // GPB        # 14 groups
CH = GPB * BLK           # 896 rows per core per group store
QB = [25, 25, 24, 24]    # blocks per quarter (per rank)
QOFF = [0, 25, 50, 74]   # quarter start block
AG_AFTER_GROUP = {3: 0, 7: 1, 10: 2, 13: 3}   # group -> quarter ready
PREFETCH_ICT = bool(int(os.environ.get("GAT_PREFETCH_ICT", "1")))

last_exec_time_ns = None


def _layout(capt):
    """Derive static ragged layout tables from per-(block,quarter) tiles."""
    lay = {}
    call_tiles = [[sum(capt[g * GPB + bb][q] for bb in range(GPB))
                   for q in range(4)] for g in range(NGRP)]
    idx_off = []
    off = 0
    for g in range(NGRP):
        for q in range(4):
            n = call_tiles[g][q] * 128
            idx_off.append((off, n))
            off += n // 16
    lay["call_tiles"] = call_tiles
    lay["idx_off"] = idx_off
    lay["idx_cols"] = off
    g3_qoff, tile_off, gtiles = [], [], []
    for g in range(NGRP):
        qo, to = [], []
        t = 0
        for q in range(4):
            qo.append(t)
            tob, tt = [], 0
            for bb in range(GPB):
                tob.append(tt)
                tt += capt[g * GPB + bb][q]
            to.append(tob)
            t += tt
        g3_qoff.append(qo)
        tile_off.append(to)
        gtiles.append(t)
    lay["g3_qoff"] = g3_qoff
    lay["tile_off"] = tile_off
    lay["gmax"] = max(gtiles)
    steps = []
    ind_col = 0
    for g in range(NGRP):
        for bg0 in range(0, GPB, 2):
            nb = min(2, GPB - bg0)
            tl = []
            for bb in range(nb):
                for q in range(4):
                    for j in range(capt[g * GPB + bg0 + bb][q]):
                        tl.append((bb, q, j))
            steps.append(dict(g=g, bg0=bg0, nb=nb, tiles=tl, nt=len(tl),
                              ind_col=ind_col))
            ind_col += 2 * len(tl) * 128
    lay["steps"] = steps
    lay["ind_cols"] = ind_col
    lay["stmax"] = max(st["nt"] for st in steps)
    return lay


def _build_program(capt, lay):
    nc = bacc.Bacc("TRN2", target_bir_lowering=False, num_swdge_queues=4)
    GMAX = lay["gmax"]
    STMAX = lay["stmax"]

    # ---- DRAM I/O ----
    d_xT = nc.dram_tensor("xT", [F_IN, NPC], BF16, kind="ExternalInput")
    d_win = nc.dram_tensor("win", [F_IN, 2 * DIM], BF16, kind="ExternalInput")
    d_bin = nc.dram_tensor("bin", [DIM, 1], F32, kind="ExternalInput")
    d_wcat = nc.dram_tensor("wcat", [DIM, 2 * L * 144], BF16, kind="ExternalInput")
    d_wskip = nc.dram_tensor("wskip", [DIM, L * DIM], BF16, kind="ExternalInput")
    d_bskip = nc.dram_tensor("bskip", [DIM, L], F32, kind="ExternalInput")
    d_idx = nc.dram_tensor("idx", [128, lay["idx_cols"]], I16, kind="ExternalInput")
    d_indc = nc.dram_tensor("indc", [128, lay["ind_cols"]], BF16, kind="ExternalInput")
    d_msk = nc.dram_tensor("msk", [128, BPC * 100], BF16, kind="ExternalInput")
    d_eyebf = nc.dram_tensor("eyebf", [128, 128], BF16, kind="ExternalInput")
    d_eye32 = nc.dram_tensor("eye32", [128, 128], F32, kind="ExternalInput")
    d_w1 = nc.dram_tensor("w1", [DIM, DIM], F32, kind="ExternalInput")
    d_w2 = nc.dram_tensor("w2", [DIM, DIM], F32, kind="ExternalInput")
    d_w3 = nc.dram_tensor("w3", [DIM, 1], F32, kind="ExternalInput")
    d_b1 = nc.dram_tensor("b1b", [128, DIM], F32, kind="ExternalInput")
    d_b2 = nc.dram_tensor("b2b", [128, DIM], F32, kind="ExternalInput")
    d_b3 = nc.dram_tensor("b3b", [128, 1], F32, kind="ExternalInput")
    d_invn = nc.dram_tensor("invn", [128, 1], F32, kind="ExternalInput")
    d_out = nc.dram_tensor("out", [100, 1], F32, kind="ExternalOutput")

    with tile.TileContext(nc) as tc:
        with (
            tc.tile_pool(name="dram", bufs=1, space="DRAM") as dram,
            tc.tile_pool(name="cst", bufs=1) as cst,
            tc.tile_pool(name="gp", bufs=2) as gp,
            tc.tile_pool(name="ic", bufs=2) as ic,
            tc.tile_pool(name="st", bufs=2) as stp,
            tc.tile_pool(name="wk", bufs=2) as wk,
            tc.tile_pool(name="pl", bufs=2, space="PSUM") as pl,
            tc.tile_pool(name="ps", bufs=2, space="PSUM") as ps,
            tc.tile_pool(name="pa", bufs=1, space="PSUM") as pa,
            tc.tile_pool(name="ph", bufs=2, space="PSUM") as ph,
            tc.tile_pool(name="pp", bufs=1, space="PSUM") as pp,
        ):
            # ---- persistent SBUF ----
            hT = cst.tile([128, NPC], BF16, tag="hT")
            idx_sb = cst.tile([128, lay["idx_cols"]], I16, tag="idx")
            eyebf_sb = cst.tile([128, 128], BF16, tag="eyebf")
            eye32_sb = cst.tile([128, 128], F32, tag="eye32")
            win_sb = cst.tile([F_IN, 2 * DIM], BF16, tag="win")
            bin_sb = cst.tile([128, 1], F32, tag="bin")
            wcat_sb = cst.tile([128, 2 * L * 144], BF16, tag="wcat")
            wskip_sb = cst.tile([128, L * DIM], BF16, tag="wskip")
            bskip_sb = cst.tile([128, L], F32, tag="bskip")
            ald_sb = [cst.tile([128, BPC * 8], BF16, tag=f"ald{j}",
                               name=f"ald{j}") for j in range(L)]

            nc.sync.dma_start(idx_sb[:], d_idx[:])
            nc.sync.dma_start(eyebf_sb[:], d_eyebf[:])
            nc.sync.dma_start(eye32_sb[:], d_eye32[:])
            nc.sync.dma_start(win_sb[:], d_win[:])
            nc.sync.dma_start(bin_sb[:], d_bin[:])
            nc.sync.dma_start(wcat_sb[:], d_wcat[:])
            nc.sync.dma_start(wskip_sb[:], d_wskip[:])
            nc.sync.dma_start(bskip_sb[:], d_bskip[:])

            nc.gpsimd.load_library(_mlp_lib)

            # ---- DRAM tiles ----
            tab_in = dram.tile([NPC, ELEM], BF16, tag="tab_in")
            tabq = [[dram.tile([NCORES * QB[q] * BLK, ELEM], BF16,
                               tag=f"tabq{j}_{q}", name=f"tabq{j}_{q}",
                               addr_space="Shared")
                     for q in range(4)] for j in range(L)]
            ar_in = dram.tile([100, DIM], F32, tag="ar_in")
            ar_out = dram.tile([100, DIM], F32, tag="ar_out", addr_space="Shared")

            def table_rows_pair(i, b0p, nb, stg, coff):
                """Table rows for nb adjacent blocks of layer i into staging."""
                pc = ph.tile([128, 288], F32, tag="hd")
                for bb in range(nb):
                    lo = (b0p + bb) * BLK
                    for part in range(2):       # hi + lo compensated weights
                        nc.tensor.matmul(
                            pc[:, bb * 144:(bb + 1) * 144],
                            lhsT=hT[:, lo:lo + BLK],
                            rhs=wcat_sb[:, (2 * i + part) * 144:
                                        (2 * i + part + 1) * 144],
                            start=(part == 0), stop=(part == 1),
                            skip_group_check=True)
                nc.scalar.activation(stg[:, coff:coff + nb * 144],
                                     pc[:, :nb * 144], AF.Copy)
                for bb in range(nb):
                    nc.scalar.activation(
                        ald_sb[i][:, (b0p + bb) * 8:(b0p + bb + 1) * 8],
                        pc[:, bb * 144 + 136:bb * 144 + 144], AF.Copy)

            def store_and_ag(i, g, stg):
                """Store staged group g rows; AllGather quarters at stage end.

                The AGs are emitted only after the last group so the CC
                traffic does not contend with the edge stage's SWDGE gather
                queues (measured: overlap degrades gather delivery 27->50ns
                per descriptor, a net loss).
                """
                dst = tab_in[g * CH:(g + 1) * CH, 0:144]
                dst3 = dst.rearrange("(b p) c -> p b c", p=128)
                src3 = stg[:].rearrange("p (b c) -> p b c", c=144)
                nc.sync.dma_start(dst3, src3)
                if g == NGRP - 1:
                    for q in range(4):
                        r0 = QOFF[q] * BLK
                        r1 = (QOFF[q] + QB[q]) * BLK
                        nc.gpsimd.collective_compute(
                            "AllGather", OP.bypass,
                            ins=[tab_in[r0:r1, :].opt()],
                            outs=[tabq[i][q][:].opt()],
                            replica_groups=[list(range(NCORES))],
                        )

            # ---- stage 0: h0 = relu(x @ W_in + b_in), table 0 fused ----
            for g in range(NGRP):
                lo = g * CH
                xc = wk.tile([F_IN, CH], BF16, tag="xc")
                nc.sync.dma_start(xc[:], d_xT[:, lo:lo + CH])
                for half in range(2):
                    w = CH // 2
                    p0 = pa.tile([128, CH // 2], F32, tag="acc")
                    nc.tensor.matmul(p0[:], lhsT=win_sb[:, 0:DIM],
                                     rhs=xc[:, half * w:(half + 1) * w],
                                     start=True, stop=False, skip_group_check=True)
                    nc.tensor.matmul(p0[:], lhsT=win_sb[:, DIM:2 * DIM],
                                     rhs=xc[:, half * w:(half + 1) * w],
                                     start=False, stop=True, skip_group_check=True)
                    nc.scalar.activation(hT[:, lo + half * w:lo + (half + 1) * w],
                                         p0[:], AF.Relu, bias=bin_sb[:, 0:1])
                stg = stp.tile([128, GPB * 144], BF16, tag="stg")
                for bb in range(0, GPB, 2):
                    nbp = min(2, GPB - bb)
                    table_rows_pair(0, g * GPB + bb, nbp, stg, bb * 144)
                store_and_ag(0, g, stg)

            # ---- layers ----
            pooled_ps = None
            for i in range(L):
                if i == L - 1:
                    pooled_ps = pp.tile([128, DIM], F32, tag="pool")
                steps = lay["steps"]
                icts = {}

                def load_ict(s, steps=steps, icts=icts):
                    st = steps[s]
                    t = ic.tile([128, 2 * STMAX * 128], BF16, tag="ICT")
                    nc.scalar.dma_start(
                        t[:, :2 * st["nt"] * 128],
                        d_indc[:, st["ind_col"]:st["ind_col"] + 2 * st["nt"] * 128])
                    icts[s] = t

                if PREFETCH_ICT:
                    load_ict(0)
                sglob = 0
                for g in range(NGRP):
                    Gt = gp.tile([128, GMAX * ELEM], BF16, tag="G")
                    G3 = Gt[:].rearrange("p (k c) -> p k c", c=ELEM)
                    for q in range(4):
                        o, n = lay["idx_off"][g * 4 + q]
                        t0 = lay["g3_qoff"][g][q]
                        ct = lay["call_tiles"][g][q]
                        nc.gpsimd.dma_gather(
                            G3[:, t0:t0 + ct, :],
                            tabq[i][q][:],
                            idx_sb[:, o:o + n // 16],
                            n, n, ELEM, single_packet=False, queue_num=q,
                        )
                    if i < L - 1:
                        stg = stp.tile([128, GPB * 144], BF16, tag="stg")
                    if i == L - 1:
                        mskb = wk.tile([128, GPB * 100], BF16, tag="mskb")
                        nc.sync.dma_start(
                            mskb[:], d_msk[:, g * GPB * 100:(g + 1) * GPB * 100])
                    for bg0 in range(0, GPB, 2):
                        st = steps[sglob]
                        nb = st["nb"]
                        nt = st["nt"]
                        tl = st["tiles"]
                        b0 = g * GPB + bg0
                        blo = b0 * BLK
                        # prefetch next step's indicator tiles; use current's
                        if PREFETCH_ICT:
                            if sglob + 1 < len(steps):
                                load_ict(sglob + 1)
                        else:
                            load_ict(sglob)
                        ICT = icts.pop(sglob)
                        sglob += 1
                        # al_d expansion on PE (IND2 tile t = cols (nt+t)*128)
                        pald = pl.tile([128, STMAX * 8], F32, tag="ald")
                        for t, (bb, q, j) in enumerate(tl):
                            nc.tensor.matmul(
                                pald[:, t * 8:(t + 1) * 8],
                                lhsT=ICT[:, (nt + t) * 128:(nt + t + 1) * 128],
                                rhs=ald_sb[i][:, (b0 + bb) * 8:(b0 + bb + 1) * 8],
                                start=True, stop=True)
                        # logits = al_s[s] + al_d[r], per (block, quarter) run
                        Lg = wk.tile([128, STMAX * 8], F32, tag="Lg")
                        pos = 0
                        for bb in range(nb):
                            for q in range(4):
                                ct = capt[b0 + bb][q]
                                gt0 = lay["g3_qoff"][g][q] + \
                                    lay["tile_off"][g][q][bg0 + bb]
                                gals = G3[:, gt0:gt0 + ct, 128:136]
                                l3 = Lg[:, pos * 8:(pos + ct) * 8].rearrange(
                                    "p (k h) -> p k h", h=8)
                                p3 = pald[:, pos * 8:(pos + ct) * 8].rearrange(
                                    "p (k h) -> p k h", h=8)
                                nc.vector.tensor_tensor(out=l3, in0=p3, in1=gals,
                                                        op=OP.add)
                                pos += ct
                        # w = exp(lrelu(logits)) fused on the scalar engine
                        Lm = wk.tile([128, STMAX * 8], F32, tag="Lm")
                        nc.scalar.activation(Lm[:, :nt * 8], Lg[:, :nt * 8],
                                             AF.Prelu, alpha=SLOPE)
                        R = wk.tile([128, STMAX * 136], BF16, tag="R")
                        R3 = R[:].rearrange("p (k c) -> p k c", c=136)
                        nc.scalar.activation(
                            R3[:, :nt, 128:136],
                            Lm[:, :nt * 8].rearrange("p (k h) -> p k h", h=8),
                            AF.Exp)
                        # contrib = hp * w, per (block, quarter) run
                        pos = 0
                        for bb in range(nb):
                            for q in range(4):
                                ct = capt[b0 + bb][q]
                                gt0 = lay["g3_qoff"][g][q] + \
                                    lay["tile_off"][g][q][bg0 + bb]
                                ghp = G3[:, gt0:gt0 + ct, 0:128].rearrange(
                                    "p k (h d) -> p k h d", d=HD)
                                rsel = R3[:, pos:pos + ct, :]
                                rw = rsel[:, :, 128:136].unsqueeze(-1) \
                                    .broadcast_to([128, ct, 8, HD])
                                rc = rsel[:, :, 0:128].rearrange(
                                    "p k (h d) -> p k h d", d=HD)
                                nc.vector.tensor_tensor(out=rc, in0=ghp, in1=rw,
                                                        op=OP.mult)
                                pos += ct
                        # segment matmuls: accumulate per block (contiguous runs)
                        pagg = pa.tile([128, 2 * 144], F32, tag="acc")
                        for t, (bb, q, j) in enumerate(tl):
                            first = (t == 0) or (tl[t - 1][0] != bb)
                            last = (t == nt - 1) or (tl[t + 1][0] != bb)
                            nc.tensor.matmul(
                                pagg[:, bb * 144:bb * 144 + 136],
                                lhsT=ICT[:, t * 128:(t + 1) * 128],
                                rhs=R[:, t * 136:(t + 1) * 136],
                                start=first, stop=last)
                        # normalize
                        rec = wk.tile([128, 16], F32, tag="rec")
                        den = pagg[:].rearrange("p (b c) -> p b c", b=2)[
                            :, :nb, 128:136]
                        rec3 = rec[:, :nb * 8].rearrange("p (b c) -> p b c", b=nb)
                        nc.vector.tensor_scalar_add(rec3, den, 1e-16)
                        nc.vector.reciprocal(rec[:, :nb * 8], rec[:, :nb * 8])
                        aggn = wk.tile([128, 2 * 128], BF16, tag="aggn")
                        for bb in range(nb):
                            nc.vector.tensor_tensor(
                                out=aggn[:, bb * 128:(bb + 1) * 128].rearrange(
                                    "p (h d) -> p h d", d=HD),
                                in0=pagg[:, bb * 144:bb * 144 + 128].rearrange(
                                    "p (h d) -> p h d", d=HD),
                                in1=rec[:, bb * 8:(bb + 1) * 8].unsqueeze(-1)
                                    .broadcast_to([128, 8, HD]),
                                op=OP.mult)
                        # skip matmul + residual on PE; bias+lrelu fused on ACT
                        phd = ph.tile([128, 288], F32, tag="hd")
                        for bb in range(nb):
                            ptn = ps.tile([128, 128], BF16, tag="sc")
                            nc.tensor.transpose(
                                ptn[:], aggn[:, bb * 128:(bb + 1) * 128],
                                eyebf_sb[:])
                            aggT = wk.tile([128, 128], BF16, tag="aggT")
                            nc.scalar.activation(aggT[:], ptn[:], AF.Copy)
                            nc.tensor.matmul(phd[:, bb * 128:(bb + 1) * 128],
                                             lhsT=wskip_sb[:, i * DIM:(i + 1) * DIM],
                                             rhs=aggT[:], start=True, stop=False,
                                             skip_group_check=True)
                            bb_lo = blo + bb * BLK
                            nc.tensor.matmul(phd[:, bb * 128:(bb + 1) * 128],
                                             lhsT=eyebf_sb[:],
                                             rhs=hT[:, bb_lo:bb_lo + BLK],
                                             start=False, stop=True,
                                             skip_group_check=True)
                        nc.scalar.activation(hT[:, blo:blo + nb * BLK],
                                             phd[:, :nb * 128],
                                             AF.Prelu, bias=bskip_sb[:, i:i + 1],
                                             alpha=SLOPE)
                        if i < L - 1:
                            table_rows_pair(i + 1, b0, nb, stg, bg0 * 144)
                        else:
                            for bb in range(nb):
                                bb_lo = blo + bb * BLK
                                b = b0 + bb
                                ptr = ps.tile([128, 128], BF16, tag="sc")
                                nc.tensor.transpose(ptr[:], hT[:, bb_lo:bb_lo + BLK],
                                                    eyebf_sb[:])
                                hrow = wk.tile([128, 128], BF16, tag="hrow")
                                nc.scalar.activation(hrow[:], ptr[:], AF.Copy)
                                nc.tensor.matmul(
                                    pooled_ps[:100, :],
                                    lhsT=mskb[:, (bg0 + bb) * 100:(bg0 + bb + 1) * 100],
                                    rhs=hrow[:], start=(b == 0),
                                    stop=(b == BPC - 1),
                                    skip_group_check=True)
                    if i < L - 1:
                        store_and_ag(i + 1, g, stg)

            # ---- pooling allreduce + MLP ----
            pooled_sb = cst.tile([128, DIM], F32, tag="pooled")
            nc.vector.memset(pooled_sb[:], 0.0)
            nc.vector.tensor_copy(pooled_sb[:100, :], pooled_ps[:100, :])
            nc.sync.dma_start(ar_in[:], pooled_sb[:100, :])
            nc.gpsimd.collective_compute(
                "AllReduce", OP.add,
                ins=[ar_in.opt()], outs=[ar_out.opt()],
                replica_groups=[list(range(NCORES))],
            )
            nc.sync.dma_start(pooled_sb[:100, :], ar_out[:])
            invn_sb = cst.tile([128, 1], F32, tag="invn")
            nc.sync.dma_start(invn_sb[:], d_invn[:])
            nc.vector.tensor_scalar_mul(pooled_sb[:], pooled_sb[:], invn_sb[:, 0:1])

            w1_sb = cst.tile([128, DIM], F32, tag="w1")
            w2_sb = cst.tile([128, DIM], F32, tag="w2")
            w3_sb = cst.tile([128, 1], F32, tag="w3")
            b1_sb = cst.tile([128, DIM], F32, tag="b1")
            b2_sb = cst.tile([128, DIM], F32, tag="b2")
            b3_sb = cst.tile([128, 1], F32, tag="b3")
            nc.sync.dma_start(w1_sb[:], d_w1[:])
            nc.sync.dma_start(w2_sb[:], d_w2[:])
            nc.sync.dma_start(w3_sb[:], d_w3[:])
            nc.sync.dma_start(b1_sb[:], d_b1[:])
            nc.sync.dma_start(b2_sb[:], d_b2[:])
            nc.sync.dma_start(b3_sb[:], d_b3[:])

            def mlp_layer(src_sb, w_sb, b_sb, ncols):
                ptz = ps.tile([128, 128], F32, tag="sc")
                nc.tensor.transpose(ptz[:], src_sb[:], eye32_sb[:])
                srcT = wk.tile([128, 128], F32, tag="srcT")
                nc.vector.tensor_copy(srcT[:], ptz[:])
                pz = pa.tile([128, 2 * 144], F32, tag="acc")
                nc.tensor.matmul(pz[:100, :ncols], lhsT=srcT[:, 0:100],
                                 rhs=w_sb[:, :ncols], start=True, stop=True)
                zo = wk.tile([128, DIM], F32, tag="zo")
                nc.vector.memset(zo[:], 0.0)
                nc.vector.tensor_tensor(out=zo[:100, :ncols], in0=pz[:100, :ncols],
                                        in1=b_sb[:100, :ncols], op=OP.add)
                z2 = wk.tile([128, DIM], F32, tag="z2")
                nc.vector.memset(z2[:], 0.0)
                nc.vector.tensor_scalar_mul(z2[:100, :ncols], zo[:100, :ncols], SLOPE)
                nc.vector.tensor_tensor(out=zo[:100, :ncols], in0=zo[:100, :ncols],
                                        in1=z2[:100, :ncols], op=OP.max)
                return zo

            z1 = mlp_layer(pooled_sb, w1_sb, b1_sb, DIM)
            z1k = cst.tile([128, DIM], F32, tag="z1k")
            nc.vector.tensor_copy(z1k[:], z1[:])
            z2 = mlp_layer(z1k, w2_sb, b2_sb, DIM)
            z2k = cst.tile([128, DIM], F32, tag="z2k")
            nc.vector.tensor_copy(z2k[:], z2[:])
            ptz = ps.tile([128, 128], F32, tag="sc")
            nc.tensor.transpose(ptz[:], z2k[:], eye32_sb[:])
            zT = wk.tile([128, 128], F32, tag="srcT")
            nc.vector.tensor_copy(zT[:], ptz[:])
            po = pa.tile([128, 2 * 144], F32, tag="acc")
            nc.tensor.matmul(po[:100, 0:1], lhsT=zT[:, 0:100], rhs=w3_sb[:],
                             start=True, stop=True)
            outp = cst.tile([128, 1], F32, tag="outp")
            nc.vector.tensor_tensor(out=outp[:100, :], in0=po[:100, 0:1],
                                    in1=b3_sb[:100, :], op=OP.add)
            nc.sync.dma_start(d_out[:], outp[:100, :])

    nc.compile()
    return nc


def _wrap_idx(flat):
    """Lay out int16 gather indices in the Q7 wrap layout for one call."""
    n = flat.shape[0]
    arr = np.zeros((16, n // 16), np.int16)
    ii = np.arange(n)
    arr[ii % 16, ii // 16] = flat.astype(np.int16)
    return np.tile(arr, (8, 1))


def _preprocess(x, senders, receivers, n_node):
    """Build per-core edge structures, capacities and input arrays."""
    order = np.argsort(receivers, kind="stable")
    r_s = receivers[order].astype(np.int64)
    s_glob = senders[order].astype(np.int64)
    # quarter tensor + row within it: [rank][blocks of quarter][128]
    s_rank = s_glob // NPC
    s_lb = (s_glob % NPC) // BLK
    s_p = s_glob % BLK
    qoff_arr = np.array(QOFF + [BPC], np.int64)
    quarter = np.digitize(s_lb, qoff_arr[1:4])
    qb_arr = np.array(QB, np.int64)
    s_s = (s_rank * qb_arr[quarter] + (s_lb - qoff_arr[quarter])) * BLK + s_p

    graph_of = np.full(NPAD, -1, np.int64)
    graph_of[:N] = np.repeat(np.arange(G), n_node.astype(np.int64))

    # first pass: bucket edges per core, get counts
    cores = []
    for c in range(NCORES):
        lo, hi = c * NPC, (c + 1) * NPC
        m = (r_s >= lo) & (r_s < hi)
        rc, sc, qc = r_s[m], s_s[m], quarter[m]
        blk = (rc - lo) // BLK
        key = blk * 4 + qc
        o2 = np.argsort(key, kind="stable")
        rc, sc, key = rc[o2], sc[o2], key[o2]
        counts = np.bincount(key, minlength=BPC * 4).reshape(BPC, 4)
        starts = np.zeros(BPC * 4 + 1, np.int64)
        np.cumsum(counts.reshape(-1), out=starts[1:])
        cores.append(dict(lo=lo, rc=rc, sc=sc, key=key, counts=counts,
                          starts=starts))

    counts_all = np.stack([cd["counts"] for cd in cores])     # [8, BPC, 4]
    capt_arr = np.maximum(1, -(-counts_all.max(axis=0) // 128))  # [BPC, 4]
    capt = capt_arr.tolist()
    lay = _layout(capt)

    arange128 = np.arange(128)
    per_core = []
    for c in range(NCORES):
        cd = cores[c]
        lo = cd["lo"]
        # ragged slot arrays
        idx_arr = np.zeros((128, lay["idx_cols"]), np.int16)
        indc = np.zeros((128, lay["ind_cols"]), "bfloat16")
        # per (b, q): slot fills
        slot_s = {}
        slot_r = {}
        for b in range(BPC):
            for q in range(4):
                cap = capt[b][q] * 128
                n = cd["counts"][b, q]
                ss = np.zeros(cap, np.int64)
                rr = np.full(cap, 128, np.int64)
                st0 = cd["starts"][b * 4 + q]
                ss[:n] = cd["sc"][st0:st0 + n]
                rr[:n] = cd["rc"][st0:st0 + n] - lo - b * BLK
                slot_s[b, q] = ss
                slot_r[b, q] = rr
        # gather index array per call (g, q)
        for g in range(NGRP):
            for q in range(4):
                o, n = lay["idx_off"][g * 4 + q]
                flat = np.concatenate(
                    [slot_s[g * GPB + bb, q] for bb in range(GPB)])
                idx_arr[:, o:o + n // 16] = _wrap_idx(flat)
        # combined per-step indicator stream [IND tiles | IND2 tiles]
        for stp_ in lay["steps"]:
            b0 = stp_["g"] * GPB + stp_["bg0"]
            nt = stp_["nt"]
            col = stp_["ind_col"]
            for t, (bb, q, j) in enumerate(stp_["tiles"]):
                rr = slot_r[b0 + bb, q][j * 128:(j + 1) * 128]
                ind = (rr[:, None] == arange128[None, :])      # [e, recv]
                indc[:, col + t * 128:col + (t + 1) * 128] = \
                    ind.astype("bfloat16")                     # IND  [e part]
                indc[:, col + (nt + t) * 128:col + (nt + t + 1) * 128] = \
                    ind.T.astype("bfloat16")                   # IND2 [recv part]
        # pooling mask
        msk = np.zeros((128, BPC * 100), np.float32)
        nodes = np.arange(lo, lo + NPC)
        gg2 = graph_of[nodes].reshape(BPC, BLK)
        for bb in range(BPC):
            valid = gg2[bb] >= 0
            msk[arange128[:BLK][valid], bb * 100 + gg2[bb][valid]] = 1.0
        xT = np.zeros((F_IN, NPC), np.float32)
        nreal = max(0, min(NPC, N - lo))
        if nreal > 0:
            xT[:, :nreal] = x[lo:lo + nreal].T
        per_core.append(dict(
            xT=xT.astype("bfloat16"),
            idx=idx_arr,
            indc=indc,
            msk=msk.astype("bfloat16"),
        ))
    return per_core, capt, lay


def _split_bf16(w):
    """Error-compensated bf16 split: w ~ hi + lo."""
    hi = w.astype("bfloat16")
    lo = (w - hi.astype(np.float32)).astype("bfloat16")
    return hi, lo


def kernel(**inputs):
    global last_exec_time_ns
    x = np.asarray(inputs["x"], np.float32)
    senders = np.asarray(inputs["senders"])
    receivers = np.asarray(inputs["receivers"])
    n_node = np.asarray(inputs["n_node"])

    per_core, capt, lay = _preprocess(x, senders, receivers, n_node)

    W_in = np.asarray(inputs["W_in"], np.float32)
    b_in = np.asarray(inputs["b_in"], np.float32)
    W_gat = np.asarray(inputs["W_gat"], np.float32)
    a_src = np.asarray(inputs["a_src"], np.float32)
    a_dst = np.asarray(inputs["a_dst"], np.float32)
    W_skip = np.asarray(inputs["W_skip"], np.float32)
    b_skip = np.asarray(inputs["b_skip"], np.float32)
    W1 = np.asarray(inputs["W1"], np.float32)
    b1 = np.asarray(inputs["b1"], np.float32)
    W2 = np.asarray(inputs["W2"], np.float32)
    b2 = np.asarray(inputs["b2"], np.float32)
    W3 = np.asarray(inputs["W3"], np.float32)
    b3 = np.asarray(inputs["b3"], np.float32)

    def w_al(Wg, a):
        A = np.zeros((DIM, H), np.float32)
        for hh in range(H):
            A[hh * HD:(hh + 1) * HD, hh] = a[hh]
        return Wg @ A

    # wcat: per layer [hi(144) | lo(144)]
    wcat_parts = []
    for i in range(L):
        wc = np.concatenate([W_gat[i], w_al(W_gat[i], a_src[i]),
                             w_al(W_gat[i], a_dst[i])], axis=1)
        hi, lo = _split_bf16(wc)
        wcat_parts += [hi, lo]
    wcat = np.concatenate(wcat_parts, axis=1)
    win_hi, win_lo = _split_bf16(W_in)
    win = np.concatenate([win_hi, win_lo], axis=1)
    wskip = np.concatenate([W_skip[i] for i in range(L)], axis=1).astype("bfloat16")
    bskip = np.stack([b_skip[i] for i in range(L)], axis=1)

    eyebf = np.eye(128, dtype=np.float32).astype("bfloat16")
    eye32 = np.eye(128, dtype=np.float32)
    b1b = np.tile(b1, (128, 1)).astype(np.float32)
    b2b = np.tile(b2, (128, 1)).astype(np.float32)
    b3b = np.full((128, 1), float(b3[0]), np.float32)
    invn = np.ones((128, 1), np.float32)
    invn[:100, 0] = 1.0 / n_node.astype(np.float32)

    shared = dict(
        win=win, bin=b_in.reshape(DIM, 1),
        wcat=wcat, wskip=wskip, bskip=bskip,
        eyebf=eyebf, eye32=eye32,
        w1=W1, w2=W2, w3=W3.reshape(DIM, 1), b1b=b1b, b2b=b2b, b3b=b3b, invn=invn,
    )

    nc = _build_program(capt, lay)
    in_maps = [{**shared, **pc} for pc in per_core]
    trace = bool(int(os.environ.get("GAT_TRACE", "0")))
    res = run_bass_kernel_spmd(nc, in_maps, core_ids=list(range(NCORES)),
                               trace=trace)
    last_exec_time_ns = res.exec_time_ns
    out = np.asarray(res.results[0]["out"], np.float32).reshape(-1)
    return out


# revision 30
# speedup vs baseline: 1.0335x; 1.0335x over previous
"""Trainium2 Bass kernel for the 3-layer GAT (nn_GAT_56341380989571).

Strategy (8 NeuronCores, SPMD):
  - Nodes padded to 100352, sharded contiguously: core k owns 12544 nodes
    (98 blocks of 128). Edges partitioned by receiver; per core, edges are
    bucketed per (128-node block, sender-quarter) with VARIABLE capacity
    (ceil(max-over-cores count / 128) tiles of 128 edge slots), so
    int16-indexed dma_gather calls (one per 7-block group x quarter,
    spread over 4 SWDGE queues) fetch per-edge rows from the quarter's
    shared table tensor.  Variable capacity cuts gather descriptors (the
    SWDGE queue-throughput bottleneck) ~25% vs fixed 3-tile buckets.
  - The per-layer node table [hp | al_s | al_d] (bf16, 512B rows) is
    exchanged via 4 CHUNKED AllGathers per layer (one per sender-quarter,
    (25,25,24,24) blocks each, into its own Shared tensor), fused into the
    edge stage of the previous layer: as soon as a 7-block group's residual
    update lands, the next layer's table rows for those blocks are
    computed, staged and stored; once all blocks of a quarter are stored
    (after groups 3/7/10/13) that quarter is AllGathered while the rest of
    the edge stage continues.
  - hp/al_s/al_d are computed as ONE bf16 matmul pair per block with
    error-compensated split weights (W ~ hi + lo in bf16, accumulated in
    the same PSUM group) to avoid the systematic bf16 weight-rounding
    bias; h itself is stored bf16 (random rounding pools away).
  - Attention: w = exp(lrelu(al_s[s]+al_d[r])) with softmax max-subtraction
    dropped and normalization folded to node level.
  - Segment sums by receiver via indicator matmuls; indicator tiles
    IND[e,p] = (r_rel[e]==p) and transposes IND2 are host-precomputed
    (layer-invariant) and streamed per step as ONE combined DMA on the
    scalar queue.  al_d is expanded edge-wise as IND2 @ al_d_block.
  - Residual add folded into the PE (identity-matmul accumulation onto the
    skip matmul); bias + leaky-relu fused into the single scalar-engine
    PSUM eviction (AF.Prelu + bias), writing h directly in bf16.
  - Graph mean-pool via a mask matmul accumulated during layer 3, then an
    AllReduce of [100,128] partial sums and a redundant tiny MLP.
"""
import os

import numpy as np

import concourse.bacc as bacc
import concourse.mybir as mybir
import concourse.tile as tile
from concourse.bass_utils import run_bass_kernel_spmd
from concourse.library_config import mlp as _mlp_lib

F32 = mybir.dt.float32
BF16 = mybir.dt.bfloat16
I16 = mybir.dt.int16
AF = mybir.ActivationFunctionType
OP = mybir.AluOpType

# problem constants (hardcoded per spec)
N, E, G = 100000, 800000, 100
F_IN, DIM, H, L = 64, 128, 8, 3
HD = DIM // H
SLOPE = 0.2
NCORES = 8
BLK = 128
BPC = 98                 # blocks per core
NPC = BPC * BLK          # 12544 nodes per core
NPAD = NCORES * NPC      # 100352
ELEM = 256               # bf16 elems per table row (512B)
GPB = 7                  # blocks per gather group
NGRP = BPC // GPB        # 14 groups
CH = GPB * BLK           # 896 rows per core per group store
QB = [25, 25, 24, 24]    # blocks per quarter (per rank)
QOFF = [0, 25, 50, 74]   # quarter start block
AG_AFTER_GROUP = {3: 0, 7: 1, 10: 2, 13: 3}   # group -> quarter ready
PREFETCH_ICT = bool(int(os.environ.get("GAT_PREFETCH_ICT", "1")))

last_exec_time_ns = None


def _layout(capt):
    """Derive static ragged layout tables from per-(block,quarter) tiles."""
    lay = {}
    call_tiles = [[sum(capt[g * GPB + bb][q] for bb in range(GPB))
                   for q in range(4)] for g in range(NGRP)]
    idx_off = []
    off = 0
    for g in range(NGRP):
        for q in range(4):
            n = call_tiles[g][q] * 128
            idx_off.append((off, n))
            off += n // 16
    lay["call_tiles"] = call_tiles
    lay["idx_off"] = idx_off
    lay["idx_cols"] = off
    g3_qoff, tile_off, gtiles = [], [], []
    for g in range(NGRP):
        qo, to = [], []
        t = 0
        for q in range(4):
            qo.append(t)
            tob, tt = [], 0
            for bb in range(GPB):
                tob.append(tt)
                tt += capt[g * GPB + bb][q]
            to.append(tob)
            t += tt
        g3_qoff.append(qo)
        tile_off.append(to)
        gtiles.append(t)
    lay["g3_qoff"] = g3_qoff
    lay["tile_off"] = tile_off
    lay["gmax"] = max(gtiles)
    steps = []
    ind_col = 0
    for g in range(NGRP):
        for bg0 in range(0, GPB, 2):
            nb = min(2, GPB - bg0)
            tl = []
            for bb in range(nb):
                for q in range(4):
                    for j in range(capt[g * GPB + bg0 + bb][q]):
                        tl.append((bb, q, j))
            steps.append(dict(g=g, bg0=bg0, nb=nb, tiles=tl, nt=len(tl),
                              ind_col=ind_col))
            ind_col += 2 * len(tl) * 128
    lay["steps"] = steps
    lay["ind_cols"] = ind_col
    lay["stmax"] = max(st["nt"] for st in steps)
    return lay


def _build_program(capt, lay):
    nc = bacc.Bacc("TRN2", target_bir_lowering=False, num_swdge_queues=4)
    GMAX = lay["gmax"]
    STMAX = lay["stmax"]

    # ---- DRAM I/O ----
    d_xT = nc.dram_tensor("xT", [F_IN, NPC], BF16, kind="ExternalInput")
    d_win = nc.dram_tensor("win", [F_IN, 2 * DIM], BF16, kind="ExternalInput")
    d_bin = nc.dram_tensor("bin", [DIM, 1], F32, kind="ExternalInput")
    d_wcat = nc.dram_tensor("wcat", [DIM, 2 * L * 144], BF16, kind="ExternalInput")
    d_wskip = nc.dram_tensor("wskip", [DIM, L * DIM], BF16, kind="ExternalInput")
    d_bskip = nc.dram_tensor("bskip", [DIM, L], F32, kind="ExternalInput")
    d_idx = nc.dram_tensor("idx", [128, lay["idx_cols"]], I16, kind="ExternalInput")
    d_indc = nc.dram_tensor("indc", [128, lay["ind_cols"]], BF16, kind="ExternalInput")
    d_msk = nc.dram_tensor("msk", [128, BPC * 100], BF16, kind="ExternalInput")
    d_eyebf = nc.dram_tensor("eyebf", [128, 128], BF16, kind="ExternalInput")
    d_eye32 = nc.dram_tensor("eye32", [128, 128], F32, kind="ExternalInput")
    d_w1 = nc.dram_tensor("w1", [DIM, DIM], F32, kind="ExternalInput")
    d_w2 = nc.dram_tensor("w2", [DIM, DIM], F32, kind="ExternalInput")
    d_w3 = nc.dram_tensor("w3", [DIM, 1], F32, kind="ExternalInput")
    d_b1 = nc.dram_tensor("b1b", [128, DIM], F32, kind="ExternalInput")
    d_b2 = nc.dram_tensor("b2b", [128, DIM], F32, kind="ExternalInput")
    d_b3 = nc.dram_tensor("b3b", [128, 1], F32, kind="ExternalInput")
    d_invn = nc.dram_tensor("invn", [128, 1], F32, kind="ExternalInput")
    d_out = nc.dram_tensor("out", [100, 1], F32, kind="ExternalOutput")

    with tile.TileContext(nc) as tc:
        with (
            tc.tile_pool(name="dram", bufs=1, space="DRAM") as dram,
            tc.tile_pool(name="cst", bufs=1) as cst,
            tc.tile_pool(name="gp", bufs=2) as gp,
            tc.tile_pool(name="ic", bufs=2) as ic,
            tc.tile_pool(name="st", bufs=2) as stp,
            tc.tile_pool(name="wk", bufs=2) as wk,
            tc.tile_pool(name="pl", bufs=2, space="PSUM") as pl,
            tc.tile_pool(name="ps", bufs=2, space="PSUM") as ps,
            tc.tile_pool(name="pa", bufs=1, space="PSUM") as pa,
            tc.tile_pool(name="ph", bufs=2, space="PSUM") as ph,
            tc.tile_pool(name="pp", bufs=1, space="PSUM") as pp,
        ):
            # ---- persistent SBUF ----
            hT = cst.tile([128, NPC], BF16, tag="hT")
            idx_sb = cst.tile([128, lay["idx_cols"]], I16, tag="idx")
            eyebf_sb = cst.tile([128, 128], BF16, tag="eyebf")
            eye32_sb = cst.tile([128, 128], F32, tag="eye32")
            win_sb = cst.tile([F_IN, 2 * DIM], BF16, tag="win")
            bin_sb = cst.tile([128, 1], F32, tag="bin")
            wcat_sb = cst.tile([128, 2 * L * 144], BF16, tag="wcat")
            wskip_sb = cst.tile([128, L * DIM], BF16, tag="wskip")
            bskip_sb = cst.tile([128, L], F32, tag="bskip")
            ald_sb = [cst.tile([128, BPC * 8], BF16, tag=f"ald{j}",
                               name=f"ald{j}") for j in range(L)]

            nc.sync.dma_start(idx_sb[:], d_idx[:])
            nc.sync.dma_start(eyebf_sb[:], d_eyebf[:])
            nc.sync.dma_start(eye32_sb[:], d_eye32[:])
            nc.sync.dma_start(win_sb[:], d_win[:])
            nc.sync.dma_start(bin_sb[:], d_bin[:])
            nc.sync.dma_start(wcat_sb[:], d_wcat[:])
            nc.sync.dma_start(wskip_sb[:], d_wskip[:])
            nc.sync.dma_start(bskip_sb[:], d_bskip[:])

            nc.gpsimd.load_library(_mlp_lib)

            # ---- DRAM tiles ----
            tab_in = dram.tile([NPC, ELEM], BF16, tag="tab_in")
            tabq = [[dram.tile([NCORES * QB[q] * BLK, ELEM], BF16,
                               tag=f"tabq{j}_{q}", name=f"tabq{j}_{q}",
                               addr_space="Shared")
                     for q in range(4)] for j in range(L)]
            ar_in = dram.tile([100, DIM], F32, tag="ar_in")
            ar_out = dram.tile([100, DIM], F32, tag="ar_out", addr_space="Shared")

            def table_rows_pair(i, b0p, nb, stg, coff):
                """Table rows for nb adjacent blocks of layer i into staging."""
                pc = ph.tile([128, 288], F32, tag="hd")
                for bb in range(nb):
                    lo = (b0p + bb) * BLK
                    for part in range(2):       # hi + lo compensated weights
                        nc.tensor.matmul(
                            pc[:, bb * 144:(bb + 1) * 144],
                            lhsT=hT[:, lo:lo + BLK],
                            rhs=wcat_sb[:, (2 * i + part) * 144:
                                        (2 * i + part + 1) * 144],
                            start=(part == 0), stop=(part == 1),
                            skip_group_check=True)
                nc.scalar.activation(stg[:, coff:coff + nb * 144],
                                     pc[:, :nb * 144], AF.Copy)
                for bb in range(nb):
                    nc.scalar.activation(
                        ald_sb[i][:, (b0p + bb) * 8:(b0p + bb + 1) * 8],
                        pc[:, bb * 144 + 136:bb * 144 + 144], AF.Copy)

            def store_and_ag(i, g, stg):
                """Store staged group g rows; AllGather quarters at stage end.

                The AGs are emitted only after the last group so the CC
                traffic does not contend with the edge stage's SWDGE gather
                queues (measured: overlap degrades gather delivery 27->50ns
                per descriptor, a net loss).
                """
                dst = tab_in[g * CH:(g + 1) * CH, 0:144]
                dst3 = dst.rearrange("(b p) c -> p b c", p=128)
                src3 = stg[:].rearrange("p (b c) -> p b c", c=144)
                nc.sync.dma_start(dst3, src3)
                if g == NGRP - 1:
                    for q in range(4):
                        r0 = QOFF[q] * BLK
                        r1 = (QOFF[q] + QB[q]) * BLK
                        nc.gpsimd.collective_compute(
                            "AllGather", OP.bypass,
                            ins=[tab_in[r0:r1, :].opt()],
                            outs=[tabq[i][q][:].opt()],
                            replica_groups=[list(range(NCORES))],
                        )

            # ---- stage 0: h0 = relu(x @ W_in + b_in), table 0 fused ----
            for g in range(NGRP):
                lo = g * CH
                xc = wk.tile([F_IN, CH], BF16, tag="xc")
                nc.sync.dma_start(xc[:], d_xT[:, lo:lo + CH])
                for half in range(2):
                    w = CH // 2
                    p0 = pa.tile([128, CH // 2], F32, tag="acc")
                    nc.tensor.matmul(p0[:], lhsT=win_sb[:, 0:DIM],
                                     rhs=xc[:, half * w:(half + 1) * w],
                                     start=True, stop=False, skip_group_check=True)
                    nc.tensor.matmul(p0[:], lhsT=win_sb[:, DIM:2 * DIM],
                                     rhs=xc[:, half * w:(half + 1) * w],
                                     start=False, stop=True, skip_group_check=True)
                    nc.scalar.activation(hT[:, lo + half * w:lo + (half + 1) * w],
                                         p0[:], AF.Relu, bias=bin_sb[:, 0:1])
                stg = stp.tile([128, GPB * 144], BF16, tag="stg")
                for bb in range(0, GPB, 2):
                    nbp = min(2, GPB - bb)
                    table_rows_pair(0, g * GPB + bb, nbp, stg, bb * 144)
                store_and_ag(0, g, stg)

            # ---- layers ----
            pooled_ps = None
            for i in range(L):
                if i == L - 1:
                    pooled_ps = pp.tile([128, DIM], F32, tag="pool")
                steps = lay["steps"]
                icts = {}

                def load_ict(s, steps=steps, icts=icts):
                    st = steps[s]
                    t = ic.tile([128, 2 * STMAX * 128], BF16, tag="ICT")
                    nc.sync.dma_start(
                        t[:, :2 * st["nt"] * 128],
                        d_indc[:, st["ind_col"]:st["ind_col"] + 2 * st["nt"] * 128])
                    icts[s] = t

                if PREFETCH_ICT:
                    load_ict(0)
                sglob = 0
                for g in range(NGRP):
                    Gt = gp.tile([128, GMAX * ELEM], BF16, tag="G")
                    G3 = Gt[:].rearrange("p (k c) -> p k c", c=ELEM)
                    for q in range(4):
                        o, n = lay["idx_off"][g * 4 + q]
                        t0 = lay["g3_qoff"][g][q]
                        ct = lay["call_tiles"][g][q]
                        nc.gpsimd.dma_gather(
                            G3[:, t0:t0 + ct, :],
                            tabq[i][q][:],
                            idx_sb[:, o:o + n // 16],
                            n, n, ELEM, single_packet=False, queue_num=q,
                        )
                    if i < L - 1:
                        stg = stp.tile([128, GPB * 144], BF16, tag="stg")
                    if i == L - 1:
                        mskb = wk.tile([128, GPB * 100], BF16, tag="mskb")
                        nc.sync.dma_start(
                            mskb[:], d_msk[:, g * GPB * 100:(g + 1) * GPB * 100])
                    for bg0 in range(0, GPB, 2):
                        st = steps[sglob]
                        nb = st["nb"]
                        nt = st["nt"]
                        tl = st["tiles"]
                        b0 = g * GPB + bg0
                        blo = b0 * BLK
                        # prefetch next step's indicator tiles; use current's
                        if PREFETCH_ICT:
                            if sglob + 1 < len(steps):
                                load_ict(sglob + 1)
                        else:
                            load_ict(sglob)
                        ICT = icts.pop(sglob)
                        sglob += 1
                        # al_d expansion on PE (IND2 tile t = cols (nt+t)*128)
                        pald = pl.tile([128, STMAX * 8], F32, tag="ald")
                        for t, (bb, q, j) in enumerate(tl):
                            nc.tensor.matmul(
                                pald[:, t * 8:(t + 1) * 8],
                                lhsT=ICT[:, (nt + t) * 128:(nt + t + 1) * 128],
                                rhs=ald_sb[i][:, (b0 + bb) * 8:(b0 + bb + 1) * 8],
                                start=True, stop=True)
                        # logits = al_s[s] + al_d[r], per (block, quarter) run
                        Lg = wk.tile([128, STMAX * 8], F32, tag="Lg")
                        pos = 0
                        for bb in range(nb):
                            for q in range(4):
                                ct = capt[b0 + bb][q]
                                gt0 = lay["g3_qoff"][g][q] + \
                                    lay["tile_off"][g][q][bg0 + bb]
                                gals = G3[:, gt0:gt0 + ct, 128:136]
                                l3 = Lg[:, pos * 8:(pos + ct) * 8].rearrange(
                                    "p (k h) -> p k h", h=8)
                                p3 = pald[:, pos * 8:(pos + ct) * 8].rearrange(
                                    "p (k h) -> p k h", h=8)
                                nc.vector.tensor_tensor(out=l3, in0=p3, in1=gals,
                                                        op=OP.add)
                                pos += ct
                        # w = exp(lrelu(logits)) fused on the scalar engine
                        Lm = wk.tile([128, STMAX * 8], F32, tag="Lm")
                        nc.scalar.activation(Lm[:, :nt * 8], Lg[:, :nt * 8],
                                             AF.Prelu, alpha=SLOPE)
                        R = wk.tile([128, STMAX * 136], BF16, tag="R")
                        R3 = R[:].rearrange("p (k c) -> p k c", c=136)
                        nc.scalar.activation(
                            R3[:, :nt, 128:136],
                            Lm[:, :nt * 8].rearrange("p (k h) -> p k h", h=8),
                            AF.Exp)
                        # contrib = hp * w, per (block, quarter) run
                        pos = 0
                        for bb in range(nb):
                            for q in range(4):
                                ct = capt[b0 + bb][q]
                                gt0 = lay["g3_qoff"][g][q] + \
                                    lay["tile_off"][g][q][bg0 + bb]
                                ghp = G3[:, gt0:gt0 + ct, 0:128].rearrange(
                                    "p k (h d) -> p k h d", d=HD)
                                rsel = R3[:, pos:pos + ct, :]
                                rw = rsel[:, :, 128:136].unsqueeze(-1) \
                                    .broadcast_to([128, ct, 8, HD])
                                rc = rsel[:, :, 0:128].rearrange(
                                    "p k (h d) -> p k h d", d=HD)
                                nc.vector.tensor_tensor(out=rc, in0=ghp, in1=rw,
                                                        op=OP.mult)
                                pos += ct
                        # segment matmuls: accumulate per block (contiguous runs)
                        pagg = pa.tile([128, 2 * 144], F32, tag="acc")
                        for t, (bb, q, j) in enumerate(tl):
                            first = (t == 0) or (tl[t - 1][0] != bb)
                            last = (t == nt - 1) or (tl[t + 1][0] != bb)
                            nc.tensor.matmul(
                                pagg[:, bb * 144:bb * 144 + 136],
                                lhsT=ICT[:, t * 128:(t + 1) * 128],
                                rhs=R[:, t * 136:(t + 1) * 136],
                                start=first, stop=last)
                        # normalize
                        rec = wk.tile([128, 16], F32, tag="rec")
                        den = pagg[:].rearrange("p (b c) -> p b c", b=2)[
                            :, :nb, 128:136]
                        rec3 = rec[:, :nb * 8].rearrange("p (b c) -> p b c", b=nb)
                        nc.vector.tensor_scalar_add(rec3, den, 1e-16)
                        nc.vector.reciprocal(rec[:, :nb * 8], rec[:, :nb * 8])
                        aggn = wk.tile([128, 2 * 128], BF16, tag="aggn")
                        for bb in range(nb):
                            nc.vector.tensor_tensor(
                                out=aggn[:, bb * 128:(bb + 1) * 128].rearrange(
                                    "p (h d) -> p h d", d=HD),
                                in0=pagg[:, bb * 144:bb * 144 + 128].rearrange(
                                    "p (h d) -> p h d", d=HD),
                                in1=rec[:, bb * 8:(bb + 1) * 8].unsqueeze(-1)
                                    .broadcast_to([128, 8, HD]),
                                op=OP.mult)
                        # skip matmul + residual on PE; bias+lrelu fused on ACT
                        phd = ph.tile([128, 288], F32, tag="hd")
                        for bb in range(nb):
                            ptn = ps.tile([128, 128], BF16, tag="sc")
                            nc.tensor.transpose(
                                ptn[:], aggn[:, bb * 128:(bb + 1) * 128],
                                eyebf_sb[:])
                            aggT = wk.tile([128, 128], BF16, tag="aggT")
                            nc.scalar.activation(aggT[:], ptn[:], AF.Copy)
                            nc.tensor.matmul(phd[:, bb * 128:(bb + 1) * 128],
                                             lhsT=wskip_sb[:, i * DIM:(i + 1) * DIM],
                                             rhs=aggT[:], start=True, stop=False,
                                             skip_group_check=True)
                            bb_lo = blo + bb * BLK
                            nc.tensor.matmul(phd[:, bb * 128:(bb + 1) * 128],
                                             lhsT=eyebf_sb[:],
                                             rhs=hT[:, bb_lo:bb_lo + BLK],
                                             start=False, stop=True,
                                             skip_group_check=True)
                        nc.scalar.activation(hT[:, blo:blo + nb * BLK],
                                             phd[:, :nb * 128],
                                             AF.Prelu, bias=bskip_sb[:, i:i + 1],
                                             alpha=SLOPE)
                        if i < L - 1:
                            table_rows_pair(i + 1, b0, nb, stg, bg0 * 144)
                        else:
                            for bb in range(nb):
                                bb_lo = blo + bb * BLK
                                b = b0 + bb
                                ptr = ps.tile([128, 128], BF16, tag="sc")
                                nc.tensor.transpose(ptr[:], hT[:, bb_lo:bb_lo + BLK],
                                                    eyebf_sb[:])
                                hrow = wk.tile([128, 128], BF16, tag="hrow")
                                nc.scalar.activation(hrow[:], ptr[:], AF.Copy)
                                nc.tensor.matmul(
                                    pooled_ps[:100, :],
                                    lhsT=mskb[:, (bg0 + bb) * 100:(bg0 + bb + 1) * 100],
                                    rhs=hrow[:], start=(b == 0),
                                    stop=(b == BPC - 1),
                                    skip_group_check=True)
                    if i < L - 1:
                        store_and_ag(i + 1, g, stg)

            # ---- pooling allreduce + MLP ----
            pooled_sb = cst.tile([128, DIM], F32, tag="pooled")
            nc.vector.memset(pooled_sb[:], 0.0)
            nc.vector.tensor_copy(pooled_sb[:100, :], pooled_ps[:100, :])
            nc.sync.dma_start(ar_in[:], pooled_sb[:100, :])
            nc.gpsimd.collective_compute(
                "AllReduce", OP.add,
                ins=[ar_in.opt()], outs=[ar_out.opt()],
                replica_groups=[list(range(NCORES))],
            )
            nc.sync.dma_start(pooled_sb[:100, :], ar_out[:])
            invn_sb = cst.tile([128, 1], F32, tag="invn")
            nc.sync.dma_start(invn_sb[:], d_invn[:])
            nc.vector.tensor_scalar_mul(pooled_sb[:], pooled_sb[:], invn_sb[:, 0:1])

            w1_sb = cst.tile([128, DIM], F32, tag="w1")
            w2_sb = cst.tile([128, DIM], F32, tag="w2")
            w3_sb = cst.tile([128, 1], F32, tag="w3")
            b1_sb = cst.tile([128, DIM], F32, tag="b1")
            b2_sb = cst.tile([128, DIM], F32, tag="b2")
            b3_sb = cst.tile([128, 1], F32, tag="b3")
            nc.sync.dma_start(w1_sb[:], d_w1[:])
            nc.sync.dma_start(w2_sb[:], d_w2[:])
            nc.sync.dma_start(w3_sb[:], d_w3[:])
            nc.sync.dma_start(b1_sb[:], d_b1[:])
            nc.sync.dma_start(b2_sb[:], d_b2[:])
            nc.sync.dma_start(b3_sb[:], d_b3[:])

            def mlp_layer(src_sb, w_sb, b_sb, ncols):
                ptz = ps.tile([128, 128], F32, tag="sc")
                nc.tensor.transpose(ptz[:], src_sb[:], eye32_sb[:])
                srcT = wk.tile([128, 128], F32, tag="srcT")
                nc.vector.tensor_copy(srcT[:], ptz[:])
                pz = pa.tile([128, 2 * 144], F32, tag="acc")
                nc.tensor.matmul(pz[:100, :ncols], lhsT=srcT[:, 0:100],
                                 rhs=w_sb[:, :ncols], start=True, stop=True)
                zo = wk.tile([128, DIM], F32, tag="zo")
                nc.vector.memset(zo[:], 0.0)
                nc.vector.tensor_tensor(out=zo[:100, :ncols], in0=pz[:100, :ncols],
                                        in1=b_sb[:100, :ncols], op=OP.add)
                z2 = wk.tile([128, DIM], F32, tag="z2")
                nc.vector.memset(z2[:], 0.0)
                nc.vector.tensor_scalar_mul(z2[:100, :ncols], zo[:100, :ncols], SLOPE)
                nc.vector.tensor_tensor(out=zo[:100, :ncols], in0=zo[:100, :ncols],
                                        in1=z2[:100, :ncols], op=OP.max)
                return zo

            z1 = mlp_layer(pooled_sb, w1_sb, b1_sb, DIM)
            z1k = cst.tile([128, DIM], F32, tag="z1k")
            nc.vector.tensor_copy(z1k[:], z1[:])
            z2 = mlp_layer(z1k, w2_sb, b2_sb, DIM)
            z2k = cst.tile([128, DIM], F32, tag="z2k")
            nc.vector.tensor_copy(z2k[:], z2[:])
            ptz = ps.tile([128, 128], F32, tag="sc")
            nc.tensor.transpose(ptz[:], z2k[:], eye32_sb[:])
            zT = wk.tile([128, 128], F32, tag="srcT")
            nc.vector.tensor_copy(zT[:], ptz[:])
            po = pa.tile([128, 2 * 144], F32, tag="acc")
            nc.tensor.matmul(po[:100, 0:1], lhsT=zT[:, 0:100], rhs=w3_sb[:],
                             start=True, stop=True)
            outp = cst.tile([128, 1], F32, tag="outp")
            nc.vector.tensor_tensor(out=outp[:100, :], in0=po[:100, 0:1],
                                    in1=b3_sb[:100, :], op=OP.add)
            nc.sync.dma_start(d_out[:], outp[:100, :])

    nc.compile()
    return nc


def _wrap_idx(flat):
    """Lay out int16 gather indices in the Q7 wrap layout for one call."""
    n = flat.shape[0]
    arr = np.zeros((16, n // 16), np.int16)
    ii = np.arange(n)
    arr[ii % 16, ii // 16] = flat.astype(np.int16)
    return np.tile(arr, (8, 1))


def _preprocess(x, senders, receivers, n_node):
    """Build per-core edge structures, capacities and input arrays."""
    order = np.argsort(receivers, kind="stable")
    r_s = receivers[order].astype(np.int64)
    s_glob = senders[order].astype(np.int64)
    # quarter tensor + row within it: [rank][blocks of quarter][128]
    s_rank = s_glob // NPC
    s_lb = (s_glob % NPC) // BLK
    s_p = s_glob % BLK
    qoff_arr = np.array(QOFF + [BPC], np.int64)
    quarter = np.digitize(s_lb, qoff_arr[1:4])
    qb_arr = np.array(QB, np.int64)
    s_s = (s_rank * qb_arr[quarter] + (s_lb - qoff_arr[quarter])) * BLK + s_p

    graph_of = np.full(NPAD, -1, np.int64)
    graph_of[:N] = np.repeat(np.arange(G), n_node.astype(np.int64))

    # first pass: bucket edges per core, get counts
    cores = []
    for c in range(NCORES):
        lo, hi = c * NPC, (c + 1) * NPC
        m = (r_s >= lo) & (r_s < hi)
        rc, sc, qc = r_s[m], s_s[m], quarter[m]
        blk = (rc - lo) // BLK
        key = blk * 4 + qc
        o2 = np.argsort(key, kind="stable")
        rc, sc, key = rc[o2], sc[o2], key[o2]
        counts = np.bincount(key, minlength=BPC * 4).reshape(BPC, 4)
        starts = np.zeros(BPC * 4 + 1, np.int64)
        np.cumsum(counts.reshape(-1), out=starts[1:])
        cores.append(dict(lo=lo, rc=rc, sc=sc, key=key, counts=counts,
                          starts=starts))

    counts_all = np.stack([cd["counts"] for cd in cores])     # [8, BPC, 4]
    capt_arr = np.maximum(1, -(-counts_all.max(axis=0) // 128))  # [BPC, 4]
    capt = capt_arr.tolist()
    lay = _layout(capt)

    arange128 = np.arange(128)
    per_core = []
    for c in range(NCORES):
        cd = cores[c]
        lo = cd["lo"]
        # ragged slot arrays
        idx_arr = np.zeros((128, lay["idx_cols"]), np.int16)
        indc = np.zeros((128, lay["ind_cols"]), "bfloat16")
        # per (b, q): slot fills
        slot_s = {}
        slot_r = {}
        for b in range(BPC):
            for q in range(4):
                cap = capt[b][q] * 128
                n = cd["counts"][b, q]
                ss = np.zeros(cap, np.int64)
                rr = np.full(cap, 128, np.int64)
                st0 = cd["starts"][b * 4 + q]
                ss[:n] = cd["sc"][st0:st0 + n]
                rr[:n] = cd["rc"][st0:st0 + n] - lo - b * BLK
                slot_s[b, q] = ss
                slot_r[b, q] = rr
        # gather index array per call (g, q)
        for g in range(NGRP):
            for q in range(4):
                o, n = lay["idx_off"][g * 4 + q]
                flat = np.concatenate(
                    [slot_s[g * GPB + bb, q] for bb in range(GPB)])
                idx_arr[:, o:o + n // 16] = _wrap_idx(flat)
        # combined per-step indicator stream [IND tiles | IND2 tiles]
        for stp_ in lay["steps"]:
            b0 = stp_["g"] * GPB + stp_["bg0"]
            nt = stp_["nt"]
            col = stp_["ind_col"]
            for t, (bb, q, j) in enumerate(stp_["tiles"]):
                rr = slot_r[b0 + bb, q][j * 128:(j + 1) * 128]
                ind = (rr[:, None] == arange128[None, :])      # [e, recv]
                indc[:, col + t * 128:col + (t + 1) * 128] = \
                    ind.astype("bfloat16")                     # IND  [e part]
                indc[:, col + (nt + t) * 128:col + (nt + t + 1) * 128] = \
                    ind.T.astype("bfloat16")                   # IND2 [recv part]
        # pooling mask
        msk = np.zeros((128, BPC * 100), np.float32)
        nodes = np.arange(lo, lo + NPC)
        gg2 = graph_of[nodes].reshape(BPC, BLK)
        for bb in range(BPC):
            valid = gg2[bb] >= 0
            msk[arange128[:BLK][valid], bb * 100 + gg2[bb][valid]] = 1.0
        xT = np.zeros((F_IN, NPC), np.float32)
        nreal = max(0, min(NPC, N - lo))
        if nreal > 0:
            xT[:, :nreal] = x[lo:lo + nreal].T
        per_core.append(dict(
            xT=xT.astype("bfloat16"),
            idx=idx_arr,
            indc=indc,
            msk=msk.astype("bfloat16"),
        ))
    return per_core, capt, lay


def _split_bf16(w):
    """Error-compensated bf16 split: w ~ hi + lo."""
    hi = w.astype("bfloat16")
    lo = (w - hi.astype(np.float32)).astype("bfloat16")
    return hi, lo


def kernel(**inputs):
    global last_exec_time_ns
    x = np.asarray(inputs["x"], np.float32)
    senders = np.asarray(inputs["senders"])
    receivers = np.asarray(inputs["receivers"])
    n_node = np.asarray(inputs["n_node"])

    per_core, capt, lay = _preprocess(x, senders, receivers, n_node)

    W_in = np.asarray(inputs["W_in"], np.float32)
    b_in = np.asarray(inputs["b_in"], np.float32)
    W_gat = np.asarray(inputs["W_gat"], np.float32)
    a_src = np.asarray(inputs["a_src"], np.float32)
    a_dst = np.asarray(inputs["a_dst"], np.float32)
    W_skip = np.asarray(inputs["W_skip"], np.float32)
    b_skip = np.asarray(inputs["b_skip"], np.float32)
    W1 = np.asarray(inputs["W1"], np.float32)
    b1 = np.asarray(inputs["b1"], np.float32)
    W2 = np.asarray(inputs["W2"], np.float32)
    b2 = np.asarray(inputs["b2"], np.float32)
    W3 = np.asarray(inputs["W3"], np.float32)
    b3 = np.asarray(inputs["b3"], np.float32)

    def w_al(Wg, a):
        A = np.zeros((DIM, H), np.float32)
        for hh in range(H):
            A[hh * HD:(hh + 1) * HD, hh] = a[hh]
        return Wg @ A

    # wcat: per layer [hi(144) | lo(144)]
    wcat_parts = []
    for i in range(L):
        wc = np.concatenate([W_gat[i], w_al(W_gat[i], a_src[i]),
                             w_al(W_gat[i], a_dst[i])], axis=1)
        hi, lo = _split_bf16(wc)
        wcat_parts += [hi, lo]
    wcat = np.concatenate(wcat_parts, axis=1)
    win_hi, win_lo = _split_bf16(W_in)
    win = np.concatenate([win_hi, win_lo], axis=1)
    wskip = np.concatenate([W_skip[i] for i in range(L)], axis=1).astype("bfloat16")
    bskip = np.stack([b_skip[i] for i in range(L)], axis=1)

    eyebf = np.eye(128, dtype=np.float32).astype("bfloat16")
    eye32 = np.eye(128, dtype=np.float32)
    b1b = np.tile(b1, (128, 1)).astype(np.float32)
    b2b = np.tile(b2, (128, 1)).astype(np.float32)
    b3b = np.full((128, 1), float(b3[0]), np.float32)
    invn = np.ones((128, 1), np.float32)
    invn[:100, 0] = 1.0 / n_node.astype(np.float32)

    shared = dict(
        win=win, bin=b_in.reshape(DIM, 1),
        wcat=wcat, wskip=wskip, bskip=bskip,
        eyebf=eyebf, eye32=eye32,
        w1=W1, w2=W2, w3=W3.reshape(DIM, 1), b1b=b1b, b2b=b2b, b3b=b3b, invn=invn,
    )

    nc = _build_program(capt, lay)
    in_maps = [{**shared, **pc} for pc in per_core]
    trace = bool(int(os.environ.get("GAT_TRACE", "0")))
    res = run_bass_kernel_spmd(nc, in_maps, core_ids=list(range(NCORES)),
                               trace=trace)
    last_exec_time_ns = res.exec_time_ns
    out = np.asarray(res.results[0]["out"], np.float32).reshape(-1)
    return out


# revision 34
# speedup vs baseline: 1.0441x; 1.0102x over previous
"""Trainium2 Bass kernel for the 3-layer GAT (nn_GAT_56341380989571).

Strategy (8 NeuronCores, SPMD):
  - Nodes padded to 100352, sharded contiguously: core k owns 12544 nodes
    (98 blocks of 128). Edges partitioned by receiver; per core, edges are
    bucketed per (128-node block, sender-quarter) with VARIABLE capacity
    (ceil(max-over-cores count / 128) tiles of 128 edge slots), so
    int16-indexed dma_gather calls (one per 7-block group x quarter,
    spread over 4 SWDGE queues) fetch per-edge rows from the quarter's
    shared table tensor.  Variable capacity cuts gather descriptors (the
    SWDGE queue-throughput bottleneck) ~25% vs fixed 3-tile buckets.
  - The per-layer node table [hp | al_s | al_d] (bf16, 512B rows) is
    exchanged via 4 CHUNKED AllGathers per layer (one per sender-quarter,
    (25,25,24,24) blocks each, into its own Shared tensor), fused into the
    edge stage of the previous layer: as soon as a 7-block group's residual
    update lands, the next layer's table rows for those blocks are
    computed, staged and stored; once all blocks of a quarter are stored
    (after groups 3/7/10/13) that quarter is AllGathered while the rest of
    the edge stage continues.
  - hp/al_s/al_d are computed as ONE bf16 matmul pair per block with
    error-compensated split weights (W ~ hi + lo in bf16, accumulated in
    the same PSUM group) to avoid the systematic bf16 weight-rounding
    bias; h itself is stored bf16 (random rounding pools away).
  - Attention: w = exp(lrelu(al_s[s]+al_d[r])) with softmax max-subtraction
    dropped and normalization folded to node level.
  - Segment sums by receiver via indicator matmuls; indicator tiles
    IND[e,p] = (r_rel[e]==p) and transposes IND2 are host-precomputed
    (layer-invariant) and streamed per step as ONE combined DMA on the
    scalar queue.  al_d is expanded edge-wise as IND2 @ al_d_block.
  - Residual add folded into the PE (identity-matmul accumulation onto the
    skip matmul); bias + leaky-relu fused into the single scalar-engine
    PSUM eviction (AF.Prelu + bias), writing h directly in bf16.
  - Graph mean-pool via a mask matmul accumulated during layer 3, then an
    AllReduce of [100,128] partial sums and a redundant tiny MLP.
"""
import os

import numpy as np

import concourse.bacc as bacc
import concourse.mybir as mybir
import concourse.tile as tile
from concourse.bass_utils import run_bass_kernel_spmd
from concourse.library_config import mlp as _mlp_lib

F32 = mybir.dt.float32
BF16 = mybir.dt.bfloat16
I16 = mybir.dt.int16
AF = mybir.ActivationFunctionType
OP = mybir.AluOpType

# problem constants (hardcoded per spec)
N, E, G = 100000, 800000, 100
F_IN, DIM, H, L = 64, 128, 8, 3
HD = DIM // H
SLOPE = 0.2
NCORES = 8
BLK = 128
BPC = 98                 # blocks per core
NPC = BPC * BLK          # 12544 nodes per core
NPAD = NCORES * NPC      # 100352
ELEM = 256               # bf16 elems per table row (512B)
GPB = 7                  # blocks per gather group
NGRP = BPC // GPB        # 14 groups
CH = GPB * BLK           # 896 rows per core per group store
QB = [25, 25, 24, 24]    # blocks per quarter (per rank)
QOFF = [0, 25, 50, 74]   # quarter start block
AG_AFTER_GROUP = {3: 0, 7: 1, 10: 2, 13: 3}   # group -> quarter ready
PREFETCH_ICT = bool(int(os.environ.get("GAT_PREFETCH_ICT", "1")))

last_exec_time_ns = None


HALF_BLOCKS = (tuple(range(0, 4)), tuple(range(4, GPB)))   # blocks per half


def _layout(capt):
    """Derive static ragged layout tables from per-(block,quarter) tiles.

    Gathers are issued per (group, half, quarter) — 8 calls per group into
    half-group G tiles — so delivery pipelines at half-group granularity.
    """
    lay = {}
    idx_off = []        # per call (g*2+h)*4+q: (idx col, num idxs)
    tile_off = []       # [g][h][q][local block]: tile offset in half tile
    call_t0 = []        # [g][h][q]: first tile of the call's segment
    off = 0
    ghmax = 0
    for g in range(NGRP):
        to_h, t0_h = [], []
        for h, bbs in enumerate(HALF_BLOCKS):
            to_q, t0_q = [], []
            t = 0
            for q in range(4):
                t0_q.append(t)
                tob = []
                for bb in bbs:
                    tob.append(t)
                    t += capt[g * GPB + bb][q]
                to_q.append(tob)
                n = (t - t0_q[q]) * 128
                idx_off.append((off, n))
                off += n // 16
            to_h.append(to_q)
            t0_h.append(t0_q)
            ghmax = max(ghmax, t)
        tile_off.append(to_h)
        call_t0.append(t0_h)
    lay["idx_off"] = idx_off
    lay["idx_cols"] = off
    lay["tile_off"] = tile_off
    lay["call_t0"] = call_t0
    lay["gmax"] = ghmax
    steps = []
    ind_col = 0
    for g in range(NGRP):
        for bg0 in range(0, GPB, 2):
            nb = min(2, GPB - bg0)
            tl = []
            for bb in range(nb):
                for q in range(4):
                    for j in range(capt[g * GPB + bg0 + bb][q]):
                        tl.append((bb, q, j))
            steps.append(dict(g=g, bg0=bg0, nb=nb, tiles=tl, nt=len(tl),
                              ind_col=ind_col))
            ind_col += 2 * len(tl) * 128
    lay["steps"] = steps
    lay["ind_cols"] = ind_col
    lay["stmax"] = max(st["nt"] for st in steps)
    return lay


def _build_program(capt, lay):
    nc = bacc.Bacc("TRN2", target_bir_lowering=False, num_swdge_queues=4)
    GMAX = lay["gmax"]
    STMAX = lay["stmax"]

    # ---- DRAM I/O ----
    d_xT = nc.dram_tensor("xT", [F_IN, NPC], BF16, kind="ExternalInput")
    d_win = nc.dram_tensor("win", [F_IN, 2 * DIM], BF16, kind="ExternalInput")
    d_bin = nc.dram_tensor("bin", [DIM, 1], F32, kind="ExternalInput")
    d_wcat = nc.dram_tensor("wcat", [DIM, 2 * L * 144], BF16, kind="ExternalInput")
    d_wskip = nc.dram_tensor("wskip", [DIM, L * DIM], BF16, kind="ExternalInput")
    d_bskip = nc.dram_tensor("bskip", [DIM, L], F32, kind="ExternalInput")
    d_idx = nc.dram_tensor("idx", [128, lay["idx_cols"]], I16, kind="ExternalInput")
    d_indc = nc.dram_tensor("indc", [128, lay["ind_cols"]], BF16, kind="ExternalInput")
    d_msk = nc.dram_tensor("msk", [128, BPC * 100], BF16, kind="ExternalInput")
    d_eyebf = nc.dram_tensor("eyebf", [128, 128], BF16, kind="ExternalInput")
    d_eye32 = nc.dram_tensor("eye32", [128, 128], F32, kind="ExternalInput")
    d_w1 = nc.dram_tensor("w1", [DIM, DIM], F32, kind="ExternalInput")
    d_w2 = nc.dram_tensor("w2", [DIM, DIM], F32, kind="ExternalInput")
    d_w3 = nc.dram_tensor("w3", [DIM, 1], F32, kind="ExternalInput")
    d_b1 = nc.dram_tensor("b1b", [128, DIM], F32, kind="ExternalInput")
    d_b2 = nc.dram_tensor("b2b", [128, DIM], F32, kind="ExternalInput")
    d_b3 = nc.dram_tensor("b3b", [128, 1], F32, kind="ExternalInput")
    d_invn = nc.dram_tensor("invn", [128, 1], F32, kind="ExternalInput")
    d_out = nc.dram_tensor("out", [100, 1], F32, kind="ExternalOutput")

    with tile.TileContext(nc) as tc:
        with (
            tc.tile_pool(name="dram", bufs=1, space="DRAM") as dram,
            tc.tile_pool(name="cst", bufs=1) as cst,
            tc.tile_pool(name="gp", bufs=3) as gp,
            tc.tile_pool(name="ic", bufs=2) as ic,
            tc.tile_pool(name="st", bufs=2) as stp,
            tc.tile_pool(name="wk", bufs=2) as wk,
            tc.tile_pool(name="pl", bufs=2, space="PSUM") as pl,
            tc.tile_pool(name="ps", bufs=2, space="PSUM") as ps,
            tc.tile_pool(name="pa", bufs=1, space="PSUM") as pa,
            tc.tile_pool(name="ph", bufs=2, space="PSUM") as ph,
            tc.tile_pool(name="pp", bufs=1, space="PSUM") as pp,
        ):
            # ---- persistent SBUF ----
            hT = cst.tile([128, NPC], BF16, tag="hT")
            idx_sb = cst.tile([128, lay["idx_cols"]], I16, tag="idx")
            eyebf_sb = cst.tile([128, 128], BF16, tag="eyebf")
            eye32_sb = cst.tile([128, 128], F32, tag="eye32")
            win_sb = cst.tile([F_IN, 2 * DIM], BF16, tag="win")
            bin_sb = cst.tile([128, 1], F32, tag="bin")
            wcat_sb = cst.tile([128, 2 * L * 144], BF16, tag="wcat")
            wskip_sb = cst.tile([128, L * DIM], BF16, tag="wskip")
            bskip_sb = cst.tile([128, L], F32, tag="bskip")
            ald_sb = [cst.tile([128, BPC * 8], BF16, tag=f"ald{j}",
                               name=f"ald{j}") for j in range(L)]

            nc.sync.dma_start(idx_sb[:], d_idx[:])
            nc.sync.dma_start(eyebf_sb[:], d_eyebf[:])
            nc.sync.dma_start(eye32_sb[:], d_eye32[:])
            nc.sync.dma_start(win_sb[:], d_win[:])
            nc.sync.dma_start(bin_sb[:], d_bin[:])
            nc.sync.dma_start(wcat_sb[:], d_wcat[:])
            nc.sync.dma_start(wskip_sb[:], d_wskip[:])
            nc.sync.dma_start(bskip_sb[:], d_bskip[:])

            nc.gpsimd.load_library(_mlp_lib)

            # ---- DRAM tiles ----
            tab_in = dram.tile([NPC, ELEM], BF16, tag="tab_in")
            tabq = [[dram.tile([NCORES * QB[q] * BLK, ELEM], BF16,
                               tag=f"tabq{j}_{q}", name=f"tabq{j}_{q}",
                               addr_space="Shared")
                     for q in range(4)] for j in range(L)]
            ar_in = dram.tile([100, DIM], F32, tag="ar_in")
            ar_out = dram.tile([100, DIM], F32, tag="ar_out", addr_space="Shared")

            def table_rows_pair(i, b0p, nb, stg, coff):
                """Table rows for nb adjacent blocks of layer i into staging."""
                pc = ph.tile([128, 288], F32, tag="hd")
                for bb in range(nb):
                    lo = (b0p + bb) * BLK
                    for part in range(2):       # hi + lo compensated weights
                        nc.tensor.matmul(
                            pc[:, bb * 144:(bb + 1) * 144],
                            lhsT=hT[:, lo:lo + BLK],
                            rhs=wcat_sb[:, (2 * i + part) * 144:
                                        (2 * i + part + 1) * 144],
                            start=(part == 0), stop=(part == 1),
                            skip_group_check=True)
                nc.scalar.activation(stg[:, coff:coff + nb * 144],
                                     pc[:, :nb * 144], AF.Copy)
                for bb in range(nb):
                    nc.scalar.activation(
                        ald_sb[i][:, (b0p + bb) * 8:(b0p + bb + 1) * 8],
                        pc[:, bb * 144 + 136:bb * 144 + 144], AF.Copy)

            def store_and_ag(i, g, stg):
                """Store staged group g rows; AllGather quarters at stage end.

                The AGs are emitted only after the last group so the CC
                traffic does not contend with the edge stage's SWDGE gather
                queues (measured: overlap degrades gather delivery 27->50ns
                per descriptor, a net loss).
                """
                dst = tab_in[g * CH:(g + 1) * CH, 0:144]
                dst3 = dst.rearrange("(b p) c -> p b c", p=128)
                src3 = stg[:].rearrange("p (b c) -> p b c", c=144)
                nc.sync.dma_start(dst3, src3)
                if g == NGRP - 1:
                    for q in range(4):
                        r0 = QOFF[q] * BLK
                        r1 = (QOFF[q] + QB[q]) * BLK
                        nc.gpsimd.collective_compute(
                            "AllGather", OP.bypass,
                            ins=[tab_in[r0:r1, :].opt()],
                            outs=[tabq[i][q][:].opt()],
                            replica_groups=[list(range(NCORES))],
                        )

            # ---- stage 0: h0 = relu(x @ W_in + b_in), table 0 fused ----
            for g in range(NGRP):
                lo = g * CH
                xc = wk.tile([F_IN, CH], BF16, tag="xc")
                nc.sync.dma_start(xc[:], d_xT[:, lo:lo + CH])
                for half in range(2):
                    w = CH // 2
                    p0 = pa.tile([128, CH // 2], F32, tag="acc")
                    nc.tensor.matmul(p0[:], lhsT=win_sb[:, 0:DIM],
                                     rhs=xc[:, half * w:(half + 1) * w],
                                     start=True, stop=False, skip_group_check=True)
                    nc.tensor.matmul(p0[:], lhsT=win_sb[:, DIM:2 * DIM],
                                     rhs=xc[:, half * w:(half + 1) * w],
                                     start=False, stop=True, skip_group_check=True)
                    nc.scalar.activation(hT[:, lo + half * w:lo + (half + 1) * w],
                                         p0[:], AF.Relu, bias=bin_sb[:, 0:1])
                stg = stp.tile([128, GPB * 144], BF16, tag="stg")
                for bb in range(0, GPB, 2):
                    nbp = min(2, GPB - bb)
                    table_rows_pair(0, g * GPB + bb, nbp, stg, bb * 144)
                store_and_ag(0, g, stg)

            # ---- layers ----
            pooled_ps = None
            for i in range(L):
                if i == L - 1:
                    pooled_ps = pp.tile([128, DIM], F32, tag="pool")
                steps = lay["steps"]
                icts = {}

                def load_ict(s, steps=steps, icts=icts):
                    st = steps[s]
                    t = ic.tile([128, 2 * STMAX * 128], BF16, tag="ICT")
                    nc.sync.dma_start(
                        t[:, :2 * st["nt"] * 128],
                        d_indc[:, st["ind_col"]:st["ind_col"] + 2 * st["nt"] * 128])
                    icts[s] = t

                if PREFETCH_ICT:
                    load_ict(0)
                sglob = 0
                for g in range(NGRP):
                    if i < L - 1:
                        stg = stp.tile([128, GPB * 144], BF16, tag="stg")
                    if i == L - 1:
                        mskb = wk.tile([128, GPB * 100], BF16, tag="mskb")
                        nc.sync.dma_start(
                            mskb[:], d_msk[:, g * GPB * 100:(g + 1) * GPB * 100])
                    for h in range(2):
                      Gt = gp.tile([128, GMAX * ELEM], BF16, tag="G")
                      G3 = Gt[:].rearrange("p (k c) -> p k c", c=ELEM)
                      for q in range(4):
                        o, n = lay["idx_off"][(g * 2 + h) * 4 + q]
                        t0 = lay["call_t0"][g][h][q]
                        nc.gpsimd.dma_gather(
                            G3[:, t0:t0 + n // 128, :],
                            tabq[i][q][:],
                            idx_sb[:, o:o + n // 16],
                            n, n, ELEM, single_packet=False, queue_num=q,
                        )
                      for bg0 in ((0, 2) if h == 0 else (4, 6)):
                        st = steps[sglob]
                        nb = st["nb"]
                        nt = st["nt"]
                        tl = st["tiles"]
                        b0 = g * GPB + bg0
                        blo = b0 * BLK
                        # prefetch next step's indicator tiles; use current's
                        if PREFETCH_ICT:
                            if sglob + 1 < len(steps):
                                load_ict(sglob + 1)
                        else:
                            load_ict(sglob)
                        ICT = icts.pop(sglob)
                        sglob += 1
                        # al_d expansion on PE (IND2 tile t = cols (nt+t)*128)
                        pald = pl.tile([128, STMAX * 8], F32, tag="ald")
                        for t, (bb, q, j) in enumerate(tl):
                            nc.tensor.matmul(
                                pald[:, t * 8:(t + 1) * 8],
                                lhsT=ICT[:, (nt + t) * 128:(nt + t + 1) * 128],
                                rhs=ald_sb[i][:, (b0 + bb) * 8:(b0 + bb + 1) * 8],
                                start=True, stop=True)
                        # logits = al_s[s] + al_d[r], per (block, quarter) run
                        Lg = wk.tile([128, STMAX * 8], F32, tag="Lg")
                        pos = 0
                        for bb in range(nb):
                            for q in range(4):
                                ct = capt[b0 + bb][q]
                                gt0 = lay["tile_off"][g][h][q][bg0 + bb - 4 * h]
                                gals = G3[:, gt0:gt0 + ct, 128:136]
                                l3 = Lg[:, pos * 8:(pos + ct) * 8].rearrange(
                                    "p (k h) -> p k h", h=8)
                                p3 = pald[:, pos * 8:(pos + ct) * 8].rearrange(
                                    "p (k h) -> p k h", h=8)
                                nc.vector.tensor_tensor(out=l3, in0=p3, in1=gals,
                                                        op=OP.add)
                                pos += ct
                        # w = exp(lrelu(logits)) fused on the scalar engine
                        Lm = wk.tile([128, STMAX * 8], F32, tag="Lm")
                        nc.scalar.activation(Lm[:, :nt * 8], Lg[:, :nt * 8],
                                             AF.Prelu, alpha=SLOPE)
                        R = wk.tile([128, STMAX * 136], BF16, tag="R")
                        R3 = R[:].rearrange("p (k c) -> p k c", c=136)
                        nc.scalar.activation(
                            R3[:, :nt, 128:136],
                            Lm[:, :nt * 8].rearrange("p (k h) -> p k h", h=8),
                            AF.Exp)
                        # contrib = hp * w, per (block, quarter) run
                        pos = 0
                        for bb in range(nb):
                            for q in range(4):
                                ct = capt[b0 + bb][q]
                                gt0 = lay["tile_off"][g][h][q][bg0 + bb - 4 * h]
                                ghp = G3[:, gt0:gt0 + ct, 0:128].rearrange(
                                    "p k (h d) -> p k h d", d=HD)
                                rsel = R3[:, pos:pos + ct, :]
                                rw = rsel[:, :, 128:136].unsqueeze(-1) \
                                    .broadcast_to([128, ct, 8, HD])
                                rc = rsel[:, :, 0:128].rearrange(
                                    "p k (h d) -> p k h d", d=HD)
                                nc.vector.tensor_tensor(out=rc, in0=ghp, in1=rw,
                                                        op=OP.mult)
                                pos += ct
                        # segment matmuls: accumulate per block (contiguous runs)
                        pagg = pa.tile([128, 2 * 144], F32, tag="acc")
                        for t, (bb, q, j) in enumerate(tl):
                            first = (t == 0) or (tl[t - 1][0] != bb)
                            last = (t == nt - 1) or (tl[t + 1][0] != bb)
                            nc.tensor.matmul(
                                pagg[:, bb * 144:bb * 144 + 136],
                                lhsT=ICT[:, t * 128:(t + 1) * 128],
                                rhs=R[:, t * 136:(t + 1) * 136],
                                start=first, stop=last)
                        # normalize
                        rec = wk.tile([128, 16], F32, tag="rec")
                        den = pagg[:].rearrange("p (b c) -> p b c", b=2)[
                            :, :nb, 128:136]
                        rec3 = rec[:, :nb * 8].rearrange("p (b c) -> p b c", b=nb)
                        nc.vector.tensor_scalar_add(rec3, den, 1e-16)
                        nc.vector.reciprocal(rec[:, :nb * 8], rec[:, :nb * 8])
                        aggn = wk.tile([128, 2 * 128], BF16, tag="aggn")
                        for bb in range(nb):
                            nc.vector.tensor_tensor(
                                out=aggn[:, bb * 128:(bb + 1) * 128].rearrange(
                                    "p (h d) -> p h d", d=HD),
                                in0=pagg[:, bb * 144:bb * 144 + 128].rearrange(
                                    "p (h d) -> p h d", d=HD),
                                in1=rec[:, bb * 8:(bb + 1) * 8].unsqueeze(-1)
                                    .broadcast_to([128, 8, HD]),
                                op=OP.mult)
                        # skip matmul + residual on PE; bias+lrelu fused on ACT
                        phd = ph.tile([128, 288], F32, tag="hd")
                        for bb in range(nb):
                            ptn = ps.tile([128, 128], BF16, tag="sc")
                            nc.tensor.transpose(
                                ptn[:], aggn[:, bb * 128:(bb + 1) * 128],
                                eyebf_sb[:])
                            aggT = wk.tile([128, 128], BF16, tag="aggT")
                            nc.scalar.activation(aggT[:], ptn[:], AF.Copy)
                            nc.tensor.matmul(phd[:, bb * 128:(bb + 1) * 128],
                                             lhsT=wskip_sb[:, i * DIM:(i + 1) * DIM],
                                             rhs=aggT[:], start=True, stop=False,
                                             skip_group_check=True)
                            bb_lo = blo + bb * BLK
                            nc.tensor.matmul(phd[:, bb * 128:(bb + 1) * 128],
                                             lhsT=eyebf_sb[:],
                                             rhs=hT[:, bb_lo:bb_lo + BLK],
                                             start=False, stop=True,
                                             skip_group_check=True)
                        nc.scalar.activation(hT[:, blo:blo + nb * BLK],
                                             phd[:, :nb * 128],
                                             AF.Prelu, bias=bskip_sb[:, i:i + 1],
                                             alpha=SLOPE)
                        if i < L - 1:
                            table_rows_pair(i + 1, b0, nb, stg, bg0 * 144)
                        else:
                            for bb in range(nb):
                                bb_lo = blo + bb * BLK
                                b = b0 + bb
                                ptr = ps.tile([128, 128], BF16, tag="sc")
                                nc.tensor.transpose(ptr[:], hT[:, bb_lo:bb_lo + BLK],
                                                    eyebf_sb[:])
                                hrow = wk.tile([128, 128], BF16, tag="hrow")
                                nc.scalar.activation(hrow[:], ptr[:], AF.Copy)
                                nc.tensor.matmul(
                                    pooled_ps[:100, :],
                                    lhsT=mskb[:, (bg0 + bb) * 100:(bg0 + bb + 1) * 100],
                                    rhs=hrow[:], start=(b == 0),
                                    stop=(b == BPC - 1),
                                    skip_group_check=True)
                    if i < L - 1:
                        store_and_ag(i + 1, g, stg)

            # ---- pooling allreduce + MLP ----
            pooled_sb = cst.tile([128, DIM], F32, tag="pooled")
            nc.vector.memset(pooled_sb[:], 0.0)
            nc.vector.tensor_copy(pooled_sb[:100, :], pooled_ps[:100, :])
            nc.sync.dma_start(ar_in[:], pooled_sb[:100, :])
            nc.gpsimd.collective_compute(
                "AllReduce", OP.add,
                ins=[ar_in.opt()], outs=[ar_out.opt()],
                replica_groups=[list(range(NCORES))],
            )
            nc.sync.dma_start(pooled_sb[:100, :], ar_out[:])
            invn_sb = cst.tile([128, 1], F32, tag="invn")
            nc.sync.dma_start(invn_sb[:], d_invn[:])
            nc.vector.tensor_scalar_mul(pooled_sb[:], pooled_sb[:], invn_sb[:, 0:1])

            w1_sb = cst.tile([128, DIM], F32, tag="w1")
            w2_sb = cst.tile([128, DIM], F32, tag="w2")
            w3_sb = cst.tile([128, 1], F32, tag="w3")
            b1_sb = cst.tile([128, DIM], F32, tag="b1")
            b2_sb = cst.tile([128, DIM], F32, tag="b2")
            b3_sb = cst.tile([128, 1], F32, tag="b3")
            nc.sync.dma_start(w1_sb[:], d_w1[:])
            nc.sync.dma_start(w2_sb[:], d_w2[:])
            nc.sync.dma_start(w3_sb[:], d_w3[:])
            nc.sync.dma_start(b1_sb[:], d_b1[:])
            nc.sync.dma_start(b2_sb[:], d_b2[:])
            nc.sync.dma_start(b3_sb[:], d_b3[:])

            def mlp_layer(src_sb, w_sb, b_sb, ncols):
                ptz = ps.tile([128, 128], F32, tag="sc")
                nc.tensor.transpose(ptz[:], src_sb[:], eye32_sb[:])
                srcT = wk.tile([128, 128], F32, tag="srcT")
                nc.vector.tensor_copy(srcT[:], ptz[:])
                pz = pa.tile([128, 2 * 144], F32, tag="acc")
                nc.tensor.matmul(pz[:100, :ncols], lhsT=srcT[:, 0:100],
                                 rhs=w_sb[:, :ncols], start=True, stop=True)
                zo = wk.tile([128, DIM], F32, tag="zo")
                nc.vector.memset(zo[:], 0.0)
                nc.vector.tensor_tensor(out=zo[:100, :ncols], in0=pz[:100, :ncols],
                                        in1=b_sb[:100, :ncols], op=OP.add)
                z2 = wk.tile([128, DIM], F32, tag="z2")
                nc.vector.memset(z2[:], 0.0)
                nc.vector.tensor_scalar_mul(z2[:100, :ncols], zo[:100, :ncols], SLOPE)
                nc.vector.tensor_tensor(out=zo[:100, :ncols], in0=zo[:100, :ncols],
                                        in1=z2[:100, :ncols], op=OP.max)
                return zo

            z1 = mlp_layer(pooled_sb, w1_sb, b1_sb, DIM)
            z1k = cst.tile([128, DIM], F32, tag="z1k")
            nc.vector.tensor_copy(z1k[:], z1[:])
            z2 = mlp_layer(z1k, w2_sb, b2_sb, DIM)
            z2k = cst.tile([128, DIM], F32, tag="z2k")
            nc.vector.tensor_copy(z2k[:], z2[:])
            ptz = ps.tile([128, 128], F32, tag="sc")
            nc.tensor.transpose(ptz[:], z2k[:], eye32_sb[:])
            zT = wk.tile([128, 128], F32, tag="srcT")
            nc.vector.tensor_copy(zT[:], ptz[:])
            po = pa.tile([128, 2 * 144], F32, tag="acc")
            nc.tensor.matmul(po[:100, 0:1], lhsT=zT[:, 0:100], rhs=w3_sb[:],
                             start=True, stop=True)
            outp = cst.tile([128, 1], F32, tag="outp")
            nc.vector.tensor_tensor(out=outp[:100, :], in0=po[:100, 0:1],
                                    in1=b3_sb[:100, :], op=OP.add)
            nc.sync.dma_start(d_out[:], outp[:100, :])

    nc.compile()
    return nc


def _wrap_idx(flat):
    """Lay out int16 gather indices in the Q7 wrap layout for one call."""
    n = flat.shape[0]
    arr = np.zeros((16, n // 16), np.int16)
    ii = np.arange(n)
    arr[ii % 16, ii // 16] = flat.astype(np.int16)
    return np.tile(arr, (8, 1))


def _preprocess(x, senders, receivers, n_node):
    """Build per-core edge structures, capacities and input arrays."""
    order = np.argsort(receivers, kind="stable")
    r_s = receivers[order].astype(np.int64)
    s_glob = senders[order].astype(np.int64)
    # quarter tensor + row within it: [rank][blocks of quarter][128]
    s_rank = s_glob // NPC
    s_lb = (s_glob % NPC) // BLK
    s_p = s_glob % BLK
    qoff_arr = np.array(QOFF + [BPC], np.int64)
    quarter = np.digitize(s_lb, qoff_arr[1:4])
    qb_arr = np.array(QB, np.int64)
    s_s = (s_rank * qb_arr[quarter] + (s_lb - qoff_arr[quarter])) * BLK + s_p

    graph_of = np.full(NPAD, -1, np.int64)
    graph_of[:N] = np.repeat(np.arange(G), n_node.astype(np.int64))

    # first pass: bucket edges per core, get counts
    cores = []
    for c in range(NCORES):
        lo, hi = c * NPC, (c + 1) * NPC
        m = (r_s >= lo) & (r_s < hi)
        rc, sc, qc = r_s[m], s_s[m], quarter[m]
        blk = (rc - lo) // BLK
        key = blk * 4 + qc
        o2 = np.argsort(key, kind="stable")
        rc, sc, key = rc[o2], sc[o2], key[o2]
        counts = np.bincount(key, minlength=BPC * 4).reshape(BPC, 4)
        starts = np.zeros(BPC * 4 + 1, np.int64)
        np.cumsum(counts.reshape(-1), out=starts[1:])
        cores.append(dict(lo=lo, rc=rc, sc=sc, key=key, counts=counts,
                          starts=starts))

    counts_all = np.stack([cd["counts"] for cd in cores])     # [8, BPC, 4]
    capt_arr = np.maximum(1, -(-counts_all.max(axis=0) // 128))  # [BPC, 4]
    capt = capt_arr.tolist()
    lay = _layout(capt)

    arange128 = np.arange(128)
    per_core = []
    for c in range(NCORES):
        cd = cores[c]
        lo = cd["lo"]
        # ragged slot arrays
        idx_arr = np.zeros((128, lay["idx_cols"]), np.int16)
        indc = np.zeros((128, lay["ind_cols"]), "bfloat16")
        # per (b, q): slot fills
        slot_s = {}
        slot_r = {}
        for b in range(BPC):
            for q in range(4):
                cap = capt[b][q] * 128
                n = cd["counts"][b, q]
                ss = np.zeros(cap, np.int64)
                rr = np.full(cap, 128, np.int64)
                st0 = cd["starts"][b * 4 + q]
                ss[:n] = cd["sc"][st0:st0 + n]
                rr[:n] = cd["rc"][st0:st0 + n] - lo - b * BLK
                slot_s[b, q] = ss
                slot_r[b, q] = rr
        # gather index array per call (g, half, q)
        for g in range(NGRP):
            for h, bbs in enumerate(HALF_BLOCKS):
                for q in range(4):
                    o, n = lay["idx_off"][(g * 2 + h) * 4 + q]
                    flat = np.concatenate(
                        [slot_s[g * GPB + bb, q] for bb in bbs])
                    idx_arr[:, o:o + n // 16] = _wrap_idx(flat)
        # combined per-step indicator stream [IND tiles | IND2 tiles]
        for stp_ in lay["steps"]:
            b0 = stp_["g"] * GPB + stp_["bg0"]
            nt = stp_["nt"]
            col = stp_["ind_col"]
            for t, (bb, q, j) in enumerate(stp_["tiles"]):
                rr = slot_r[b0 + bb, q][j * 128:(j + 1) * 128]
                ind = (rr[:, None] == arange128[None, :])      # [e, recv]
                indc[:, col + t * 128:col + (t + 1) * 128] = \
                    ind.astype("bfloat16")                     # IND  [e part]
                indc[:, col + (nt + t) * 128:col + (nt + t + 1) * 128] = \
                    ind.T.astype("bfloat16")                   # IND2 [recv part]
        # pooling mask
        msk = np.zeros((128, BPC * 100), np.float32)
        nodes = np.arange(lo, lo + NPC)
        gg2 = graph_of[nodes].reshape(BPC, BLK)
        for bb in range(BPC):
            valid = gg2[bb] >= 0
            msk[arange128[:BLK][valid], bb * 100 + gg2[bb][valid]] = 1.0
        xT = np.zeros((F_IN, NPC), np.float32)
        nreal = max(0, min(NPC, N - lo))
        if nreal > 0:
            xT[:, :nreal] = x[lo:lo + nreal].T
        per_core.append(dict(
            xT=xT.astype("bfloat16"),
            idx=idx_arr,
            indc=indc,
            msk=msk.astype("bfloat16"),
        ))
    return per_core, capt, lay


def _split_bf16(w):
    """Error-compensated bf16 split: w ~ hi + lo."""
    hi = w.astype("bfloat16")
    lo = (w - hi.astype(np.float32)).astype("bfloat16")
    return hi, lo


def kernel(**inputs):
    global last_exec_time_ns
    x = np.asarray(inputs["x"], np.float32)
    senders = np.asarray(inputs["senders"])
    receivers = np.asarray(inputs["receivers"])
    n_node = np.asarray(inputs["n_node"])

    per_core, capt, lay = _preprocess(x, senders, receivers, n_node)

    W_in = np.asarray(inputs["W_in"], np.float32)
    b_in = np.asarray(inputs["b_in"], np.float32)
    W_gat = np.asarray(inputs["W_gat"], np.float32)
    a_src = np.asarray(inputs["a_src"], np.float32)
    a_dst = np.asarray(inputs["a_dst"], np.float32)
    W_skip = np.asarray(inputs["W_skip"], np.float32)
    b_skip = np.asarray(inputs["b_skip"], np.float32)
    W1 = np.asarray(inputs["W1"], np.float32)
    b1 = np.asarray(inputs["b1"], np.float32)
    W2 = np.asarray(inputs["W2"], np.float32)
    b2 = np.asarray(inputs["b2"], np.float32)
    W3 = np.asarray(inputs["W3"], np.float32)
    b3 = np.asarray(inputs["b3"], np.float32)

    def w_al(Wg, a):
        A = np.zeros((DIM, H), np.float32)
        for hh in range(H):
            A[hh * HD:(hh + 1) * HD, hh] = a[hh]
        return Wg @ A

    # wcat: per layer [hi(144) | lo(144)]
    wcat_parts = []
    for i in range(L):
        wc = np.concatenate([W_gat[i], w_al(W_gat[i], a_src[i]),
                             w_al(W_gat[i], a_dst[i])], axis=1)
        hi, lo = _split_bf16(wc)
        wcat_parts += [hi, lo]
    wcat = np.concatenate(wcat_parts, axis=1)
    win_hi, win_lo = _split_bf16(W_in)
    win = np.concatenate([win_hi, win_lo], axis=1)
    wskip = np.concatenate([W_skip[i] for i in range(L)], axis=1).astype("bfloat16")
    bskip = np.stack([b_skip[i] for i in range(L)], axis=1)

    eyebf = np.eye(128, dtype=np.float32).astype("bfloat16")
    eye32 = np.eye(128, dtype=np.float32)
    b1b = np.tile(b1, (128, 1)).astype(np.float32)
    b2b = np.tile(b2, (128, 1)).astype(np.float32)
    b3b = np.full((128, 1), float(b3[0]), np.float32)
    invn = np.ones((128, 1), np.float32)
    invn[:100, 0] = 1.0 / n_node.astype(np.float32)

    shared = dict(
        win=win, bin=b_in.reshape(DIM, 1),
        wcat=wcat, wskip=wskip, bskip=bskip,
        eyebf=eyebf, eye32=eye32,
        w1=W1, w2=W2, w3=W3.reshape(DIM, 1), b1b=b1b, b2b=b2b, b3b=b3b, invn=invn,
    )

    nc = _build_program(capt, lay)
    in_maps = [{**shared, **pc} for pc in per_core]
    trace = bool(int(os.environ.get("GAT_TRACE", "0")))
    res = run_bass_kernel_spmd(nc, in_maps, core_ids=list(range(NCORES)),
                               trace=trace)
    last_exec_time_ns = res.exec_time_ns
    out = np.asarray(res.results[0]["out"], np.float32).reshape(-1)
    return out
